# revision 43
# baseline (speedup 1.0000x reference)
"""Trainium2 Bass kernel for nn_Decoder_22196390985918 (SPADE-style decoder).

Sharding: 8 cores = (batch b in 0..3) x (H-half in 0..1). Each core computes
out[b, :, h0:h0+64, :] for h0 = 64*(core%2).

v2 restructure vs baseline:
- All weights host-side pre-transposed into final lhsT layouts, blend
  (sigmoid) factors folded in on host, cast to bf16, packed into 2 big
  DMA blobs + 1 small fp32 const blob -> ~6 input DMAs total instead of ~60.
- x loaded once as bf16 [128, 8192]: partitions 0-63 = own 64 rows
  (channel-major, reused by the epilogue), 64-127 = other half (stats only).
  Instance-norm stats via one multi-chunk bn_stats; halves combined with
  plain [64]-partition tensor ops (no transposes / DMAs).
- Region-priority mask fused: sel = (cnt == 0) * seg as one DVE
  scalar_tensor_tensor reading cnt straight from PSUM.
- sel45 partition order (j, ty, tx) so the G table rearrange is ONE
  SBUF->SBUF DMA. u5 = kron(tril, eye(9)) accordingly.
- Epilogue: out = (x - mu) * A + B with A = (psum_g + bias_g)*rstd via one
  ACT op, B = psum_b + bias_b via another; no PE broadcast matmul.
- Main conv loop starts as soon as selG is ready (~20us) and overlaps all
  remaining DMA.
"""
import numpy as np
import ml_dtypes

import concourse.bacc as bacc
import concourse.bass as bass
import concourse.mybir as mybir
import concourse.tile as tile
from concourse.bass_utils import run_bass_kernel_spmd

dt = mybir.dt
F32 = dt.float32
BF16 = dt.bfloat16
AF = mybir.ActivationFunctionType
ALU = mybir.AluOpType
BF = ml_dtypes.bfloat16

B, C, H, W, F, L, NH = 4, 64, 128, 128, 5, 512, 128
GW = 130                    # padded grid width  (image col = grid col - 1)
SR = 66                     # seg/sel/actv grid rows (image row = h0 - 1 + r)
MR = 68                     # mask grid rows (image row = h0 - 2 + r)
SEG_N = SR * GW             # 8580
MASK_N = MR * GW            # 8840
GLS = SEG_N + 48            # seg grid line length (incl. u5 tail)
GLM = MASK_N                # mask grid line length
ROWS = 64                   # output rows per core
NCH = 16                    # main conv chunks (4 rows x 128 cols, N=512)
ACH = 22                    # shared conv chunks (3 rows x 128 cols, N=384)
NCORES = 8

# bigwa layout (per-partition elem offsets): fcw [5*4*512] then codes [5*512]
OFF_FCW = 0
OFF_CODES = 5 * 4 * 512     # 10240
BWA = OFF_CODES + 5 * 512   # 12800
# bigwb layout: wct [4*9*128] then spT [9*128] then sswT [128]
OFF_WCT = 0
OFF_SPT = 4 * 9 * 128       # 4608
OFF_SSW = OFF_SPT + 9 * 128  # 5760
BWB = OFF_SSW + 128         # 5888
# constf layout (fp32): 0 biasg, 1 ssb, 2-3 hal, 4-23 fcbt, 24 biasb64
CF = 25
# u5 lives in the grids blob tail on partitions 0..44
OFF_U5 = SEG_N              # 8580..8625


def _build_nc():
    nc = bacc.Bacc()

    gseg_d = nc.dram_tensor("gseg", [46, SEG_N], BF16, kind="ExternalInput")
    gmask_d = nc.dram_tensor("gmask", [27, MASK_N], BF16,
                             kind="ExternalInput")
    u5_d = nc.dram_tensor("u5d", [45, 48], BF16, kind="ExternalInput")
    brow_d = nc.dram_tensor("brow", [1, 128], BF16, kind="ExternalInput")
    spt8_d = nc.dram_tensor("spt8", [128, 1152], dt.float8e4,
                            kind="ExternalInput")
    bigwa_d = nc.dram_tensor("bigwa", [128, BWA], BF16, kind="ExternalInput")
    bigwb_d = nc.dram_tensor("bigwb", [128, BWB], BF16, kind="ExternalInput")
    constf_d = nc.dram_tensor("constf", [128, CF], F32, kind="ExternalInput")
    xb2_d = nc.dram_tensor("xb2", [128, 8192], BF16, kind="ExternalInput")
    out_d = nc.dram_tensor("out", [C, 4, 4 * 512], BF16, kind="ExternalOutput")

    with tile.TileContext(nc) as tc:
        with (
            tc.tile_pool(name="const", bufs=1) as cst,
            tc.tile_pool(name="mus", bufs=2) as musp,
            tc.tile_pool(name="gb", bufs=3) as gbp,
            tc.tile_pool(name="ep", bufs=3) as epp,
            tc.tile_pool(name="ot", bufs=2) as otp,
            tc.tile_pool(name="pmain", bufs=4, space="PSUM") as pmain,
            tc.tile_pool(name="paux", bufs=2, space="PSUM") as paux,
            tc.tile_pool(name="pg", bufs=2, space="PSUM") as pgp,
        ):
            # ---- input DMAs ---------------------------------------------
            segc = cst.tile([16, SEG_N + 2 * GW + 4], BF16)
            nc.sync.dma_start(out=segc[:], in_=segc_d[:])
            maskc = cst.tile([9, MASK_N + 2 * GW + 4], BF16)
            nc.sync.dma_start(out=maskc[:], in_=maskc_d[:])
            u5t = cst.tile([45, 48], BF16)
            nc.sync.dma_start(out=u5t[:], in_=u5_d[:])
            spt8 = cst.tile([128, 4, 2, 128], dt.float8e4)
            spt8b = cst.tile([128, 128], dt.float8e4)
            nc.sync.dma_start(out=spt8[:], in_=spt8_d[:, 0:1024].rearrange(
                "p (a s c) -> p a s c", a=4, s=2))
            nc.sync.dma_start(out=spt8b[:], in_=spt8_d[:, 1024:1152])
            constf = cst.tile([128, CF], F32)
            nc.gpsimd.dma_start(out=constf[:], in_=constf_d[:])
            bigwa = cst.tile([128, BWA], BF16)
            nc.gpsimd.dma_start(out=bigwa[:], in_=bigwa_d[:])
            bigwb = cst.tile([128, BWB], BF16)
            nc.sync.dma_start(out=bigwb[:], in_=bigwb_d[:])
            xb2 = cst.tile([128, 8192], BF16)
            nc.gpsimd.dma_start(out=xb2[:], in_=xb2_d[:])

            # on-chip 9-shift replication (vector DMA queue, SBUF->SBUF)
            gseg = cst.tile([46, SEG_N], BF16)
            sca = segc[:]
            nc.vector.dma_start(
                out=gseg[0:45, :],
                in_=bass.AP(tensor=sca.tensor, offset=sca.offset,
                            ap=[sca.ap[0][:1] + [5], [GW, 3], [1, 3],
                                [1, SEG_N]]))
            nc.vector.dma_start(out=gseg[45:46, :], in_=segc[5:6, 0:SEG_N])
            gmask = cst.tile([27, MASK_N], BF16)
            mca = maskc[:]
            nc.vector.dma_start(
                out=gmask[:],
                in_=bass.AP(tensor=mca.tensor, offset=mca.offset,
                            ap=[mca.ap[0][:1] + [3], [GW, 3], [1, 3],
                                [1, MASK_N]]))

            sel45 = gseg[0:45, 0:SEG_N]
            u5 = u5t[:, 0:45]
            mask27 = gmask[:, 0:MASK_N]
            epst = cst.tile([128, 1], F32)
            nc.gpsimd.memset(epst[:], 1e-5)
            zt = cst.tile([128, 1], F32)
            nc.gpsimd.memset(zt[:], 0.0)
            biasg = constf[:, 0:1]
            ssb = constf[:, 1:2]
            hal = constf[:, 2:4]
            fcbt = constf[:, 4:24].rearrange("p (k j) -> p k j", k=4)
            biasb64 = constf[0:64, 24:25]

            # ---- region masks: cnt (PE) -> sel = (cnt==0)*seg (DVE) -----
            segchunks = []
            off = 0
            while off < SEG_N:
                n = min(512, SEG_N - off)
                segchunks.append((off, n))
                off += n
            for off, n in segchunks:
                pc = paux.tile([45, 512], F32, tag="aux")
                nc.tensor.matmul(pc[:, 0:n], u5, sel45[:, off:off + n],
                                 start=True, stop=True)
                nc.scalar.activation(sel45[:, off:off + n], pc[:, 0:n],
                                     AF.Relu, bias=zt[0:45, :])

            # ---- shared conv (mask 3 -> NH) + actv assembly -------------
            actv = cst.tile([NH, SR, GW], dt.float8e4)
            bord = actv[:, :, 0:1]
            nc.gpsimd.memset(
                bass.AP(tensor=bord.tensor, offset=bord.offset,
                        ap=[bord.ap[0], [GW, SR], [GW - 1, 2]]), 0.0)
            sswT = bigwb[0:27, OFF_SSW:OFF_SSW + 128]
            m3 = mask27.rearrange("p (r c) -> p r c", c=GW)
            for a in range(ACH):
                r = 3 * a
                psh = paux.tile([NH, 3, 128], F32, tag="aux")
                nc.tensor.matmul(psh[:], sswT, m3[:, r:r + 3, 0:128],
                                 start=True, stop=True)
                nc.scalar.activation(actv[:, r:r + 3, 1:129], psh[:], AF.Relu,
                                     bias=ssb, scale=1.0)
            nc.scalar.activation(actv[:, 0, :], actv[:, 0, :], AF.Identity,
                                  bias=zt[:], scale=hal[:, 0:1])
            nc.scalar.activation(actv[:, SR - 1, :], actv[:, SR - 1, :],
                                 AF.Identity, bias=zt[:], scale=hal[:, 1:2])

            # ---- mu path: z = fcb + sum_l fcw*codes (DVE TTR), relu -----
            z_sb = cst.tile([128, 4, F], F32)
            for j in range(F):
                cview = bigwa[:, OFF_CODES + j * L:OFF_CODES + (j + 1) * L]
                for kb in range(4):
                    fview = bigwa[:, OFF_FCW + (j * 4 + kb) * L:
                                  OFF_FCW + (j * 4 + kb + 1) * L]
                    mus = musp.tile([128, L], BF16, tag="mus")
                    nc.vector.scalar_tensor_tensor(
                        out=mus[:], in0=fview, scalar=1.0, in1=cview,
                        op0=ALU.mult, op1=ALU.mult,
                        accum_out=z_sb[:, kb, j:j + 1])
            nc.vector.tensor_add(z_sb[:], z_sb[:], fcbt[:])
            muT = cst.tile([128, 4, F], BF16)
            nc.scalar.activation(muT[:], z_sb[:], AF.Relu, bias=zt[:])

            # ---- G tables -> selG (one rearrange DMA) -------------------
            gstage = cst.tile([F, 9, 128], BF16)
            for g in range(3):
                gps = pgp.tile([F, 3, 128], F32, tag="gps")
                for kb in range(4):
                    wview = bigwb[:, OFF_WCT + kb * 1152 + g * 384:
                                  OFF_WCT + kb * 1152 + (g + 1) * 384]
                    nc.tensor.matmul(gps[:], muT[:, kb, :],
                                     wview.rearrange("p (t c) -> p t c", t=3),
                                     start=(kb == 0), stop=(kb == 3))
                nc.scalar.activation(gstage[:, 3 * g:3 * g + 3, :], gps[:], AF.Copy)
            selG = cst.tile([46, 128], BF16)
            for ty in range(3):
                nc.scalar.dma_start(out=selG[15 * ty:15 * ty + 15, :],
                                    in_=gstage[:, 3 * ty:3 * ty + 3, :])
            nc.scalar.dma_start(out=selG[45:46, :], in_=brow_d[:])

            # ---- instance-norm stats ------------------------------------
            st = cst.tile([128, 16, 6], F32)
            for q in range(16):
                nc.vector.bn_stats(out=st[:, q, :],
                                   in_=xb2[:, q * 512:(q + 1) * 512])
            mv = cst.tile([128, 2], F32)
            nc.vector.bn_aggr(out=mv[:], in_=st[:])
            mvm = cst.tile([128, 2], F32)
            nc.vector.tensor_copy(mvm[:, 0:1], mv[:, 0:1])
            nc.vector.scalar_tensor_tensor(
                out=mvm[:, 1:2], in0=mv[:, 0:1], scalar=mv[:, 0:1], in1=mv[:, 1:2],
                op0=ALU.mult, op1=ALU.add)
            oth = cst.tile([64, 2], F32)
            nc.vector.stream_shuffle(oth[:], mvm[64:128, :], list(range(32)))
            mus_ = cst.tile([64, 1], F32)
            nc.vector.tensor_add(mus_[:], mvm[0:64, 0:1], oth[:, 0:1])
            m2s = cst.tile([64, 1], F32)
            nc.vector.tensor_add(m2s[:], mvm[0:64, 1:2], oth[:, 1:2])
            muc = cst.tile([64, 1], F32)
            nc.vector.tensor_scalar_mul(muc[:], mus_[:], 0.5)
            mu2 = cst.tile([64, 1], F32)
            nc.vector.tensor_mul(mu2[:], muc[:], muc[:])
            varc = cst.tile([64, 1], F32)
            nc.vector.scalar_tensor_tensor(
                out=varc[:], in0=m2s[:], scalar=0.5, in1=mu2[:],
                op0=ALU.mult, op1=ALU.subtract)
            sd = cst.tile([64, 1], F32)
            nc.scalar.activation(sd[:], varc[:], AF.Sqrt, bias=epst[0:64, :])
            rstd = cst.tile([64, 1], F32)
            nc.vector.reciprocal(rstd[:], sd[:])
            rstd64 = cst.tile([64, 1], F32)
            nc.vector.tensor_scalar_mul(rstd64[:], rstd[:], 1.0 / 64.0)

            # ---- main conv + epilogue (epilogue one chunk behind) -------
            s3 = gseg[:].rearrange("p (r c) -> p r c", c=GW)
            a3 = actv[:]
            pms = {}
            ots = {}

            aoff = [ty * GW + tx for ty in range(3) for tx in range(3)]
            a8 = actv[:]

            def conv_chunk(i):
                pm = pmain.tile([128, 4, 128], F32, tag="pm", name=f"pm_{i}")
                pms[i] = pm
                for P in range(4):
                    o0, o1 = aoff[2 * P], aoff[2 * P + 1]
                    rhs = bass.AP(
                        tensor=a8.tensor,
                        offset=a8.offset + 4 * i * GW + o0,
                        ap=[a8.ap[0], [o1 - o0, 2], [GW, 4], [1, 128]])
                    nc.tensor.matmul(pm[:], spt8[:, P, :, :], rhs,
                                     start=(P == 0), stop=False,
                                     perf_mode=mybir.MatmulPerfMode.DoubleRow,
                                     skip_group_check=True)
                rhs8 = bass.AP(tensor=a8.tensor,
                               offset=a8.offset + 4 * i * GW + aoff[8],
                               ap=[a8.ap[0], [GW, 4], [1, 128]])
                nc.tensor.matmul(pm[:], spt8b[:], rhs8, start=False, stop=False,
                                 skip_group_check=True)
                nc.tensor.matmul(pm[:], selG[:], s3[:, 4 * i:4 * i + 4, 0:128],
                                 start=False, stop=True, skip_group_check=True)

            def epi_chunk(i):
                pm = pms.pop(i)
                if i % 4 == 0:
                    ots[i // 4] = otp.tile([64, 4, 4, 128], BF16, tag="ot",
                                           name=f"ot_{i // 4}")
                ot = ots[i // 4]
                gt = gbp.tile([128, 4, 128], F32, tag="gb")
                nc.scalar.activation(gt[0:64, :, :], pm[0:64, :, :], AF.Identity,
                                     bias=zt[0:64, :], scale=rstd64[:])
                nc.scalar.activation(gt[64:128, :, :], pm[64:128, :, :],
                                     AF.Identity, bias=0.0, scale=1.0 / 64.0)
                ga = gt[0:64, :, :]
                bbm = epp.tile([64, 4, 128], F32, tag="ep")
                nc.vector.stream_shuffle(bbm[:], gt[64:128, :, :], list(range(32)))
                xa = epp.tile([64, 4, 128], F32, tag="ep")
                nc.vector.scalar_tensor_tensor(
                    out=xa[:], in0=xb2[0:64, i * 512:(i + 1) * 512].rearrange(
                        "p (r w) -> p r w", r=4),
                    scalar=muc[:], in1=ga, op0=ALU.subtract,
                    op1=ALU.mult)
                nc.gpsimd.tensor_add(ot[:, i % 4, :, :], xa[:], bbm[:])
                if i % 4 == 3:
                    q = i // 4
                    nc.sync.dma_start(
                        out=out_d[:, q, :],
                        in_=ots.pop(q)[:].rearrange("c k r w -> c (k r w)"))

            conv_chunk(0)
            for i in range(1, NCH):
                conv_chunk(i)
                epi_chunk(i - 1)
            epi_chunk(NCH - 1)

    nc.finalize()
    return nc


_NC = None


def kernel(**inputs):
    global _NC
    x = np.asarray(inputs["x"], dtype=np.float32)
    segmap = np.asarray(inputs["segmap"], dtype=np.float32)
    codes_vector = np.asarray(inputs["codes_vector"], dtype=np.float32)
    mask = np.asarray(inputs["mask"], dtype=np.float32)
    fc_w = np.asarray(inputs["fc_w"], dtype=np.float32)
    fc_b = np.asarray(inputs["fc_b"], dtype=np.float32)
    cgw = np.asarray(inputs["conv_gamma_w"], dtype=np.float32)
    cgb = np.asarray(inputs["conv_gamma_b"], dtype=np.float32)
    cbw = np.asarray(inputs["conv_beta_w"], dtype=np.float32)
    cbb = np.asarray(inputs["conv_beta_b"], dtype=np.float32)
    ssw = np.asarray(inputs["spade_shared_w"], dtype=np.float32)
    ssb = np.asarray(inputs["spade_shared_b"], dtype=np.float32)
    sgw = np.asarray(inputs["spade_gamma_w"], dtype=np.float32)
    sgb = np.asarray(inputs["spade_gamma_b"], dtype=np.float32)
    sbw = np.asarray(inputs["spade_beta_w"], dtype=np.float32)
    sbb = np.asarray(inputs["spade_beta_b"], dtype=np.float32)
    bg = float(np.asarray(inputs["blending_gamma"]).reshape(-1)[0])
    bb_ = float(np.asarray(inputs["blending_beta"]).reshape(-1)[0])

    if _NC is None:
        _NC = _build_nc()

    ga = 1.0 / (1.0 + np.exp(-bg))
    ba = 1.0 / (1.0 + np.exp(-bb_))

    # bigwb: wct | spT | sswT  (shared across cores)
    bigwb = np.zeros((128, BWB), np.float32)
    # wct[p, kb*1152 + (3ty+tx)*128 + cc] = blend * conv_w[cc, kb*128+p, ty, tx]
    cw = np.concatenate([cgw * ga, cbw * ba], axis=0) * 64.0   # [128, 512, 3, 3]
    wct = cw.reshape(128, 4, 128, 9).transpose(2, 1, 3, 0)     # [p, kb, t, cc]
    bigwb[:, OFF_WCT:OFF_SPT] = wct.reshape(128, 4608)
    sw = np.concatenate([sgw * (1 - ga), sbw * (1 - ba)], axis=0)  # [128, NH, 3, 3]
    spT = sw.reshape(128, 128, 9).transpose(1, 2, 0) * 64.0    # [nh, t, cc]
    spt8h = np.zeros((128, 1152), np.float32)
    spt8h[:, 0:1024] = spT[:, 0:8, :].reshape(128, 1024)
    spt8h[:, 1024:1152] = spT[:, 8, :]
    spt8h = np.ascontiguousarray(spt8h.astype(ml_dtypes.float8_e4m3))
    # sswT[9ty+3c+tx, nh] = ssw[nh, c, ty, tx] (on-chip mask replicate order)
    sswT = ssw.transpose(2, 1, 3, 0).reshape(27, 128)
    bigwb[0:27, OFF_SSW:OFF_SSW + 128] = sswT
    bigwb = bigwb.astype(BF)

    # constf: biasg | ssb | hal | fcbt (hal per-core, rest shared)
    constf_base = np.zeros((128, CF), np.float32)
    constf_base[0:64, 0] = ga * cgb + (1 - ga) * sgb + 1.0
    constf_base[64:128, 0] = ba * cbb + (1 - ba) * sbb
    constf_base[:, 1] = ssb
    constf_base[0:64, 24] = ba * cbb + (1 - ba) * sbb
    # fcbt[p, 4 + kb*5 + j] = fc_b[j, kb*128+p]
    constf_base[:, 4:24] = fc_b.T.reshape(4, 128, F).transpose(1, 0, 2).reshape(128, 20)

    # u5[9j'+t', 9j+t] = (j' > j) * (t'==t)
    u5 = (np.eye(45, dtype=np.float32)
          - np.kron(np.eye(3, dtype=np.float32),
                    np.kron(np.tril(np.ones((F, F), np.float32), -1),
                            np.eye(3, dtype=np.float32))))
    brow = np.zeros((1, 128), np.float32)
    brow[0, 0:64] = (ga * cgb + (1 - ga) * sgb + 1.0) * 64.0
    brow[0, 64:128] = (ba * cbb + (1 - ba) * sbb) * 64.0
    brow = brow.astype(BF)
    u5h = np.zeros((45, 48), np.float32)
    u5h[:, 0:45] = u5
    u5h = np.ascontiguousarray(u5h.astype(BF))

    in_maps = []
    for c in range(NCORES):
        b, half = divmod(c, 2)
        h0 = half * ROWS

        # bigwa: fcw (k-partition, l-free) | codes broadcast (per-batch)
        bigwa = np.zeros((128, BWA), np.float32)
        # fcw_sec[p, (j*4+kb)*512 + l] = fc_w[j, kb*128+p, l]
        bigwa[:, OFF_FCW:OFF_CODES] = (
            fc_w.reshape(F, 4, 128, L).transpose(2, 0, 1, 3).reshape(128, 10240))
        bigwa[:, OFF_CODES:BWA] = np.broadcast_to(
            codes_vector[b].reshape(1, F * L), (128, F * L))

        gsegh = np.ones((46, SEG_N), np.float32)
        gmaskh = np.zeros((27, MASK_N), np.float32)
        segp = np.zeros((F, SR + 2, GW + 2), np.float32)
        r_lo, r_hi = h0 - 1, h0 + ROWS + 1
        s_lo, s_hi = max(r_lo, 0), min(r_hi, H)
        segp[:, s_lo - r_lo:s_hi - r_lo, 1:129] = segmap[b, :, s_lo:s_hi, :]
        for ty in range(3):
            for j in range(F):
                for tx in range(3):
                    gsegh[15 * ty + 3 * j + tx, :] = (
                        segp[j, ty:ty + SR, tx:tx + GW].reshape(-1))
        maskp = np.zeros((3, MR + 2, GW + 2), np.float32)
        m_lo, m_hi = h0 - 2, h0 + ROWS + 2
        ms_lo, ms_hi = max(m_lo, 0), min(m_hi, H)
        maskp[:, ms_lo - m_lo:ms_hi - m_lo, 1:129] = mask[b, :, ms_lo:ms_hi, :]
        for ty in range(3):
            for cc in range(3):
                for tx in range(3):
                    gmaskh[9 * ty + 3 * cc + tx, :] = (
                        maskp[cc, ty:ty + MR, tx:tx + GW].reshape(-1))

        constf = constf_base.copy()
        constf[:, 2] = 0.0 if h0 == 0 else 1.0
        constf[:, 3] = 0.0 if h0 + ROWS == H else 1.0

        xb2 = np.concatenate([
            x[b, :, h0:h0 + ROWS, :].reshape(C, 8192),
            x[b, :, ROWS - h0:ROWS - h0 + ROWS, :].reshape(C, 8192)], axis=0)

        in_maps.append(dict(
            brow=brow,
            spt8=spt8h,
            u5d=u5h,
            gseg=np.ascontiguousarray(gsegh.astype(BF)),
            gmask=np.ascontiguousarray(gmaskh.astype(BF)),
            bigwa=np.ascontiguousarray(bigwa.astype(BF)),
            bigwb=bigwb,
            constf=np.ascontiguousarray(constf),
            xb2=np.ascontiguousarray(xb2.astype(BF)),
        ))

    res = run_bass_kernel_spmd(_NC, in_maps, list(range(NCORES)))

    out = np.empty((B, C, H, W), np.float32)
    for c in range(NCORES):
        b, half = divmod(c, 2)
        h0 = half * ROWS
        out[b, :, h0:h0 + ROWS, :] = res.results[c]["out"].astype(
            np.float32).reshape(C, ROWS, W)
    return out


# revision 45
# speedup vs baseline: 1.0120x; 1.0120x over previous
"""Trainium2 Bass kernel for nn_Decoder_22196390985918 (SPADE-style decoder).

Sharding: 8 cores = (batch b in 0..3) x (H-half in 0..1). Each core computes
out[b, :, h0:h0+64, :] for h0 = 64*(core%2).

v2 restructure vs baseline:
- All weights host-side pre-transposed into final lhsT layouts, blend
  (sigmoid) factors folded in on host, cast to bf16, packed into 2 big
  DMA blobs + 1 small fp32 const blob -> ~6 input DMAs total instead of ~60.
- x loaded once as bf16 [128, 8192]: partitions 0-63 = own 64 rows
  (channel-major, reused by the epilogue), 64-127 = other half (stats only).
  Instance-norm stats via one multi-chunk bn_stats; halves combined with
  plain [64]-partition tensor ops (no transposes / DMAs).
- Region-priority mask fused: sel = (cnt == 0) * seg as one DVE
  scalar_tensor_tensor reading cnt straight from PSUM.
- sel45 partition order (j, ty, tx) so the G table rearrange is ONE
  SBUF->SBUF DMA. u5 = kron(tril, eye(9)) accordingly.
- Epilogue: out = (x - mu) * A + B with A = (psum_g + bias_g)*rstd via one
  ACT op, B = psum_b + bias_b via another; no PE broadcast matmul.
- Main conv loop starts as soon as selG is ready (~20us) and overlaps all
  remaining DMA.
"""
import numpy as np
import ml_dtypes

import concourse.bacc as bacc
import concourse.bass as bass
import concourse.mybir as mybir
import concourse.tile as tile
from concourse.bass_utils import run_bass_kernel_spmd

dt = mybir.dt
F32 = dt.float32
BF16 = dt.bfloat16
AF = mybir.ActivationFunctionType
ALU = mybir.AluOpType
BF = ml_dtypes.bfloat16

B, C, H, W, F, L, NH = 4, 64, 128, 128, 5, 512, 128
GW = 130                    # padded grid width  (image col = grid col - 1)
SR = 66                     # seg/sel/actv grid rows (image row = h0 - 1 + r)
MR = 68                     # mask grid rows (image row = h0 - 2 + r)
SEG_N = SR * GW             # 8580
MASK_N = MR * GW            # 8840
GLS = SEG_N + 48            # seg grid line length (incl. u5 tail)
GLM = MASK_N                # mask grid line length
ROWS = 64                   # output rows per core
NCH = 16                    # main conv chunks (4 rows x 128 cols, N=512)
ACH = 22                    # shared conv chunks (3 rows x 128 cols, N=384)
NCORES = 8

# bigwa layout (per-partition elem offsets): fcw [5*4*512] then codes [5*512]
OFF_FCW = 0
OFF_CODES = 5 * 4 * 512     # 10240
BWA = OFF_CODES + 5 * 512   # 12800
# bigwb layout: wct [4*9*128] then spT [9*128] then sswT [128]
OFF_WCT = 0
OFF_SPT = 4 * 9 * 128       # 4608
OFF_SSW = OFF_SPT + 9 * 128  # 5760
BWB = OFF_SSW + 128         # 5888
# constf layout (fp32): 0 biasg, 1 ssb, 2-3 hal, 4-23 fcbt, 24 biasb64
CF = 25
# u5 lives in the grids blob tail on partitions 0..44
OFF_U5 = SEG_N              # 8580..8625


def _build_nc():
    nc = bacc.Bacc()

    gseg_d = nc.dram_tensor("gseg", [46, SEG_N], BF16, kind="ExternalInput")
    gmask_d = nc.dram_tensor("gmask", [27, MASK_N], BF16,
                             kind="ExternalInput")
    u5_d = nc.dram_tensor("u5d", [45, 48], BF16, kind="ExternalInput")
    brow_d = nc.dram_tensor("brow", [1, 128], BF16, kind="ExternalInput")
    spt8_d = nc.dram_tensor("spt8", [128, 1152], dt.float8e4,
                            kind="ExternalInput")
    bigwa_d = nc.dram_tensor("bigwa", [128, BWA], BF16, kind="ExternalInput")
    bigwb_d = nc.dram_tensor("bigwb", [128, BWB], BF16, kind="ExternalInput")
    constf_d = nc.dram_tensor("constf", [128, CF], F32, kind="ExternalInput")
    xb2_d = nc.dram_tensor("xb2", [128, 8192], BF16, kind="ExternalInput")
    out_d = nc.dram_tensor("out", [C, 4, 4 * 512], BF16, kind="ExternalOutput")

    with tile.TileContext(nc) as tc:
        with (
            tc.tile_pool(name="const", bufs=1) as cst,
            tc.tile_pool(name="mus", bufs=2) as musp,
            tc.tile_pool(name="gb", bufs=3) as gbp,
            tc.tile_pool(name="ep", bufs=3) as epp,
            tc.tile_pool(name="ot", bufs=2) as otp,
            tc.tile_pool(name="pmain", bufs=4, space="PSUM") as pmain,
            tc.tile_pool(name="paux", bufs=2, space="PSUM") as paux,
            tc.tile_pool(name="pg", bufs=2, space="PSUM") as pgp,
        ):
            # ---- input DMAs ---------------------------------------------
            segc = cst.tile([16, SEG_N + 2 * GW + 4], BF16)
            nc.sync.dma_start(out=segc[:], in_=segc_d[:])
            maskc = cst.tile([9, MASK_N + 2 * GW + 4], BF16)
            nc.sync.dma_start(out=maskc[:], in_=maskc_d[:])
            u5t = cst.tile([45, 48], BF16)
            nc.sync.dma_start(out=u5t[:], in_=u5_d[:])
            spt8 = cst.tile([128, 4, 2, 128], dt.float8e4)
            spt8b = cst.tile([128, 128], dt.float8e4)
            constf = cst.tile([128, CF], F32)
            nc.gpsimd.dma_start(out=constf[:], in_=constf_d[:])
            bigwa = cst.tile([128, BWA], BF16)
            nc.gpsimd.dma_start(out=bigwa[:], in_=bigwa_d[:])
            bigwb = cst.tile([128, BWB], BF16)
            nc.sync.dma_start(out=bigwb[:], in_=bigwb_d[:])
            xb2 = cst.tile([128, 8192], BF16)
            nc.gpsimd.dma_start(out=xb2[:], in_=xb2_d[:])

            # on-chip 9-shift replication (vector DMA queue, SBUF->SBUF)
            gseg = cst.tile([46, SEG_N], BF16)
            sca = segc[:]
            nc.vector.dma_start(
                out=gseg[0:45, :],
                in_=bass.AP(tensor=sca.tensor, offset=sca.offset,
                            ap=[sca.ap[0][:1] + [5], [GW, 3], [1, 3],
                                [1, SEG_N]]))
            nc.vector.dma_start(out=gseg[45:46, :], in_=segc[5:6, 0:SEG_N])
            gmask = cst.tile([27, MASK_N], BF16)
            mca = maskc[:]
            nc.vector.dma_start(
                out=gmask[:],
                in_=bass.AP(tensor=mca.tensor, offset=mca.offset,
                            ap=[mca.ap[0][:1] + [3], [GW, 3], [1, 3],
                                [1, MASK_N]]))

            sel45 = gseg[0:45, 0:SEG_N]
            u5 = u5t[:, 0:45]
            mask27 = gmask[:, 0:MASK_N]
            epst = cst.tile([128, 1], F32)
            nc.gpsimd.memset(epst[:], 1e-5)
            zt = cst.tile([128, 1], F32)
            nc.gpsimd.memset(zt[:], 0.0)
            biasg = constf[:, 0:1]
            ssb = constf[:, 1:2]
            hal = constf[:, 2:4]
            fcbt = constf[:, 4:24].rearrange("p (k j) -> p k j", k=4)
            biasb64 = constf[0:64, 24:25]

            # ---- region masks: cnt (PE) -> sel = (cnt==0)*seg (DVE) -----
            segchunks = []
            off = 0
            while off < SEG_N:
                n = min(512, SEG_N - off)
                segchunks.append((off, n))
                off += n
            for off, n in segchunks:
                pc = paux.tile([45, 512], F32, tag="aux")
                nc.tensor.matmul(pc[:, 0:n], u5, sel45[:, off:off + n],
                                 start=True, stop=True)
                nc.scalar.activation(sel45[:, off:off + n], pc[:, 0:n],
                                     AF.Relu, bias=zt[0:45, :])

            # ---- shared conv (mask 3 -> NH) + actv assembly -------------
            actv = cst.tile([NH, SR, GW], dt.float8e4)
            bord = actv[:, :, 0:1]
            nc.gpsimd.memset(
                bass.AP(tensor=bord.tensor, offset=bord.offset,
                        ap=[bord.ap[0], [GW, SR], [GW - 1, 2]]), 0.0)
            sswT = bigwb[0:27, OFF_SSW:OFF_SSW + 128]
            m3 = mask27.rearrange("p (r c) -> p r c", c=GW)
            for a in range(ACH):
                r = 3 * a
                psh = paux.tile([NH, 3, 128], F32, tag="aux")
                nc.tensor.matmul(psh[:], sswT, m3[:, r:r + 3, 0:128],
                                 start=True, stop=True)
                nc.scalar.activation(actv[:, r:r + 3, 1:129], psh[:], AF.Relu,
                                     bias=ssb, scale=1.0)
            nc.scalar.activation(actv[:, 0, :], actv[:, 0, :], AF.Identity,
                                  bias=zt[:], scale=hal[:, 0:1])
            nc.scalar.activation(actv[:, SR - 1, :], actv[:, SR - 1, :],
                                 AF.Identity, bias=zt[:], scale=hal[:, 1:2])

            # ---- mu path: z = fcb + sum_l fcw*codes (DVE TTR), relu -----
            z_sb = cst.tile([128, 4, F], F32)
            for j in range(F):
                cview = bigwa[:, OFF_CODES + j * L:OFF_CODES + (j + 1) * L]
                for kb in range(4):
                    fview = bigwa[:, OFF_FCW + (j * 4 + kb) * L:
                                  OFF_FCW + (j * 4 + kb + 1) * L]
                    mus = musp.tile([128, L], BF16, tag="mus")
                    nc.vector.scalar_tensor_tensor(
                        out=mus[:], in0=fview, scalar=1.0, in1=cview,
                        op0=ALU.mult, op1=ALU.mult,
                        accum_out=z_sb[:, kb, j:j + 1])
            nc.vector.tensor_add(z_sb[:], z_sb[:], fcbt[:])
            muT = cst.tile([128, 4, F], BF16)
            nc.scalar.activation(muT[:], z_sb[:], AF.Relu, bias=zt[:])

            # ---- G tables -> selG (one rearrange DMA) -------------------
            gstage = cst.tile([F, 9, 128], BF16)
            for g in range(3):
                gps = pgp.tile([F, 3, 128], F32, tag="gps")
                for kb in range(4):
                    wview = bigwb[:, OFF_WCT + kb * 1152 + g * 384:
                                  OFF_WCT + kb * 1152 + (g + 1) * 384]
                    nc.tensor.matmul(gps[:], muT[:, kb, :],
                                     wview.rearrange("p (t c) -> p t c", t=3),
                                     start=(kb == 0), stop=(kb == 3))
                nc.scalar.activation(gstage[:, 3 * g:3 * g + 3, :], gps[:], AF.Copy)
            selG = cst.tile([46, 128], BF16)
            for ty in range(3):
                nc.scalar.dma_start(out=selG[15 * ty:15 * ty + 15, :],
                                    in_=gstage[:, 3 * ty:3 * ty + 3, :])
            nc.scalar.dma_start(out=selG[45:46, :], in_=brow_d[:])

            # ---- instance-norm stats ------------------------------------
            st = cst.tile([128, 16, 6], F32)
            for q in range(16):
                nc.vector.bn_stats(out=st[:, q, :],
                                   in_=xb2[:, q * 512:(q + 1) * 512])
            mv = cst.tile([128, 2], F32)
            nc.vector.bn_aggr(out=mv[:], in_=st[:])
            mvm = cst.tile([128, 2], F32)
            nc.vector.tensor_copy(mvm[:, 0:1], mv[:, 0:1])
            nc.vector.scalar_tensor_tensor(
                out=mvm[:, 1:2], in0=mv[:, 0:1], scalar=mv[:, 0:1], in1=mv[:, 1:2],
                op0=ALU.mult, op1=ALU.add)
            oth = cst.tile([64, 2], F32)
            nc.vector.stream_shuffle(oth[:], mvm[64:128, :], list(range(32)))
            mus_ = cst.tile([64, 1], F32)
            nc.vector.tensor_add(mus_[:], mvm[0:64, 0:1], oth[:, 0:1])
            m2s = cst.tile([64, 1], F32)
            nc.vector.tensor_add(m2s[:], mvm[0:64, 1:2], oth[:, 1:2])
            muc = cst.tile([64, 1], F32)
            nc.vector.tensor_scalar_mul(muc[:], mus_[:], 0.5)
            mu2 = cst.tile([64, 1], F32)
            nc.vector.tensor_mul(mu2[:], muc[:], muc[:])
            varc = cst.tile([64, 1], F32)
            nc.vector.scalar_tensor_tensor(
                out=varc[:], in0=m2s[:], scalar=0.5, in1=mu2[:],
                op0=ALU.mult, op1=ALU.subtract)
            ve = cst.tile([64, 1], F32)
            nc.vector.tensor_scalar_add(ve[:], varc[:], 1e-5)
            ri = cst.tile([64, 1], dt.int32)
            nc.vector.tensor_scalar(ri[:], ve[:].bitcast(dt.int32),
                                    1, None, op0=ALU.arith_shift_right)
            nc.vector.tensor_scalar(ri[:], ri[:], 0x5f3759df, -1,
                                    op0=ALU.subtract, op1=ALU.mult)
            r0 = cst.tile([64, 1], F32)
            nc.vector.tensor_copy(r0[:], ri[:].bitcast(F32))
            rstd = cst.tile([64, 1], F32)
            ra = cst.tile([64, 1], F32)
            rb = cst.tile([64, 1], F32)
            for it in range(2):
                nc.vector.tensor_mul(ra[:], r0[:], r0[:])
                nc.vector.tensor_mul(rb[:], ra[:], ve[:])
                nc.vector.tensor_scalar(rb[:], rb[:], -0.5, 1.5,
                                        op0=ALU.mult, op1=ALU.add)
                nc.vector.tensor_mul(r0[:], r0[:], rb[:])
            nc.vector.tensor_copy(rstd[:], r0[:])
            rstd64 = cst.tile([64, 1], F32)
            nc.vector.tensor_scalar_mul(rstd64[:], rstd[:], 1.0 / 64.0)

            # ---- main conv + epilogue (epilogue one chunk behind) -------
            s3 = gseg[:].rearrange("p (r c) -> p r c", c=GW)
            a3 = actv[:]
            pms = {}
            ots = {}

            aoff = [ty * GW + tx for ty in range(3) for tx in range(3)]
            a8 = actv[:]

            def conv_chunk(i):
                pm = pmain.tile([128, 4, 128], F32, tag="pm", name=f"pm_{i}")
                pms[i] = pm
                for P in range(4):
                    o0, o1 = aoff[2 * P], aoff[2 * P + 1]
                    rhs = bass.AP(
                        tensor=a8.tensor,
                        offset=a8.offset + 4 * i * GW + o0,
                        ap=[a8.ap[0], [o1 - o0, 2], [GW, 4], [1, 128]])
                    nc.tensor.matmul(pm[:], spt8[:, P, :, :], rhs,
                                     start=(P == 0), stop=False,
                                     perf_mode=mybir.MatmulPerfMode.DoubleRow,
                                     skip_group_check=True)
                rhs8 = bass.AP(tensor=a8.tensor,
                               offset=a8.offset + 4 * i * GW + aoff[8],
                               ap=[a8.ap[0], [GW, 4], [1, 128]])
                nc.tensor.matmul(pm[:], spt8b[:], rhs8, start=False, stop=False,
                                 skip_group_check=True)
                nc.tensor.matmul(pm[:], selG[:], s3[:, 4 * i:4 * i + 4, 0:128],
                                 start=False, stop=True, skip_group_check=True)

            def epi_chunk(i):
                pm = pms.pop(i)
                if i % 4 == 0:
                    ots[i // 4] = otp.tile([64, 4, 4, 128], BF16, tag="ot",
                                           name=f"ot_{i // 4}")
                ot = ots[i // 4]
                gt = gbp.tile([128, 4, 128], F32, tag="gb")
                nc.scalar.activation(gt[0:64, :, :], pm[0:64, :, :], AF.Identity,
                                     bias=zt[0:64, :], scale=rstd64[:])
                nc.scalar.activation(gt[64:128, :, :], pm[64:128, :, :],
                                     AF.Identity, bias=0.0, scale=1.0 / 64.0)
                ga = gt[0:64, :, :]
                bbm = epp.tile([64, 4, 128], F32, tag="ep")
                nc.vector.stream_shuffle(bbm[:], gt[64:128, :, :], list(range(32)))
                xa = epp.tile([64, 4, 128], F32, tag="ep")
                nc.vector.scalar_tensor_tensor(
                    out=xa[:], in0=xb2[0:64, i * 512:(i + 1) * 512].rearrange(
                        "p (r w) -> p r w", r=4),
                    scalar=muc[:], in1=ga, op0=ALU.subtract,
                    op1=ALU.mult)
                nc.gpsimd.tensor_add(ot[:, i % 4, :, :], xa[:], bbm[:])
                if i % 4 == 3:
                    q = i // 4
                    nc.sync.dma_start(
                        out=out_d[:, q, :],
                        in_=ots.pop(q)[:].rearrange("c k r w -> c (k r w)"))

            conv_chunk(0)
            for i in range(1, NCH):
                conv_chunk(i)
                epi_chunk(i - 1)
            epi_chunk(NCH - 1)

    nc.finalize()
    return nc


_NC = None


def kernel(**inputs):
    global _NC
    x = np.asarray(inputs["x"], dtype=np.float32)
    segmap = np.asarray(inputs["segmap"], dtype=np.float32)
    codes_vector = np.asarray(inputs["codes_vector"], dtype=np.float32)
    mask = np.asarray(inputs["mask"], dtype=np.float32)
    fc_w = np.asarray(inputs["fc_w"], dtype=np.float32)
    fc_b = np.asarray(inputs["fc_b"], dtype=np.float32)
    cgw = np.asarray(inputs["conv_gamma_w"], dtype=np.float32)
    cgb = np.asarray(inputs["conv_gamma_b"], dtype=np.float32)
    cbw = np.asarray(inputs["conv_beta_w"], dtype=np.float32)
    cbb = np.asarray(inputs["conv_beta_b"], dtype=np.float32)
    ssw = np.asarray(inputs["spade_shared_w"], dtype=np.float32)
    ssb = np.asarray(inputs["spade_shared_b"], dtype=np.float32)
    sgw = np.asarray(inputs["spade_gamma_w"], dtype=np.float32)
    sgb = np.asarray(inputs["spade_gamma_b"], dtype=np.float32)
    sbw = np.asarray(inputs["spade_beta_w"], dtype=np.float32)
    sbb = np.asarray(inputs["spade_beta_b"], dtype=np.float32)
    bg = float(np.asarray(inputs["blending_gamma"]).reshape(-1)[0])
    bb_ = float(np.asarray(inputs["blending_beta"]).reshape(-1)[0])

    if _NC is None:
        _NC = _build_nc()

    ga = 1.0 / (1.0 + np.exp(-bg))
    ba = 1.0 / (1.0 + np.exp(-bb_))

    # bigwb: wct | spT | sswT  (shared across cores)
    bigwb = np.zeros((128, BWB), np.float32)
    # wct[p, kb*1152 + (3ty+tx)*128 + cc] = blend * conv_w[cc, kb*128+p, ty, tx]
    cw = np.concatenate([cgw * ga, cbw * ba], axis=0) * 64.0   # [128, 512, 3, 3]
    wct = cw.reshape(128, 4, 128, 9).transpose(2, 1, 3, 0)     # [p, kb, t, cc]
    bigwb[:, OFF_WCT:OFF_SPT] = wct.reshape(128, 4608)
    sw = np.concatenate([sgw * (1 - ga), sbw * (1 - ba)], axis=0)  # [128, NH, 3, 3]
    spT = sw.reshape(128, 128, 9).transpose(1, 2, 0) * 64.0    # [nh, t, cc]
    spt8h = np.zeros((128, 1152), np.float32)
    spt8h[:, 0:1024] = spT[:, 0:8, :].reshape(128, 1024)
    spt8h[:, 1024:1152] = spT[:, 8, :]
    spt8h = np.ascontiguousarray(spt8h.astype(ml_dtypes.float8_e4m3))
    # sswT[9ty+3c+tx, nh] = ssw[nh, c, ty, tx] (on-chip mask replicate order)
    sswT = ssw.transpose(2, 1, 3, 0).reshape(27, 128)
    bigwb[0:27, OFF_SSW:OFF_SSW + 128] = sswT
    bigwb = bigwb.astype(BF)

    # constf: biasg | ssb | hal | fcbt (hal per-core, rest shared)
    constf_base = np.zeros((128, CF), np.float32)
    constf_base[0:64, 0] = ga * cgb + (1 - ga) * sgb + 1.0
    constf_base[64:128, 0] = ba * cbb + (1 - ba) * sbb
    constf_base[:, 1] = ssb
    constf_base[0:64, 24] = ba * cbb + (1 - ba) * sbb
    # fcbt[p, 4 + kb*5 + j] = fc_b[j, kb*128+p]
    constf_base[:, 4:24] = fc_b.T.reshape(4, 128, F).transpose(1, 0, 2).reshape(128, 20)

    # u5[9j'+t', 9j+t] = (j' > j) * (t'==t)
    u5 = (np.eye(45, dtype=np.float32)
          - np.kron(np.eye(3, dtype=np.float32),
                    np.kron(np.tril(np.ones((F, F), np.float32), -1),
                            np.eye(3, dtype=np.float32))))
    brow = np.zeros((1, 128), np.float32)
    brow[0, 0:64] = (ga * cgb + (1 - ga) * sgb + 1.0) * 64.0
    brow[0, 64:128] = (ba * cbb + (1 - ba) * sbb) * 64.0
    brow = brow.astype(BF)
    u5h = np.zeros((45, 48), np.float32)
    u5h[:, 0:45] = u5
    u5h = np.ascontiguousarray(u5h.astype(BF))

    in_maps = []
    for c in range(NCORES):
        b, half = divmod(c, 2)
        h0 = half * ROWS

        # bigwa: fcw (k-partition, l-free) | codes broadcast (per-batch)
        bigwa = np.zeros((128, BWA), np.float32)
        # fcw_sec[p, (j*4+kb)*512 + l] = fc_w[j, kb*128+p, l]
        bigwa[:, OFF_FCW:OFF_CODES] = (
            fc_w.reshape(F, 4, 128, L).transpose(2, 0, 1, 3).reshape(128, 10240))
        bigwa[:, OFF_CODES:BWA] = np.broadcast_to(
            codes_vector[b].reshape(1, F * L), (128, F * L))

        gsegh = np.ones((46, SEG_N), np.float32)
        gmaskh = np.zeros((27, MASK_N), np.float32)
        segp = np.zeros((F, SR + 2, GW + 2), np.float32)
        r_lo, r_hi = h0 - 1, h0 + ROWS + 1
        s_lo, s_hi = max(r_lo, 0), min(r_hi, H)
        segp[:, s_lo - r_lo:s_hi - r_lo, 1:129] = segmap[b, :, s_lo:s_hi, :]
        for ty in range(3):
            for j in range(F):
                for tx in range(3):
                    gsegh[15 * ty + 3 * j + tx, :] = (
                        segp[j, ty:ty + SR, tx:tx + GW].reshape(-1))
        maskp = np.zeros((3, MR + 2, GW + 2), np.float32)
        m_lo, m_hi = h0 - 2, h0 + ROWS + 2
        ms_lo, ms_hi = max(m_lo, 0), min(m_hi, H)
        maskp[:, ms_lo - m_lo:ms_hi - m_lo, 1:129] = mask[b, :, ms_lo:ms_hi, :]
        for ty in range(3):
            for cc in range(3):
                for tx in range(3):
                    gmaskh[9 * ty + 3 * cc + tx, :] = (
                        maskp[cc, ty:ty + MR, tx:tx + GW].reshape(-1))

        constf = constf_base.copy()
        constf[:, 2] = 0.0 if h0 == 0 else 1.0
        constf[:, 3] = 0.0 if h0 + ROWS == H else 1.0

        xb2 = np.concatenate([
            x[b, :, h0:h0 + ROWS, :].reshape(C, 8192),
            x[b, :, ROWS - h0:ROWS - h0 + ROWS, :].reshape(C, 8192)], axis=0)

        in_maps.append(dict(
            brow=brow,
            spt8=spt8h,
            u5d=u5h,
            gseg=np.ascontiguousarray(gsegh.astype(BF)),
            gmask=np.ascontiguousarray(gmaskh.astype(BF)),
            bigwa=np.ascontiguousarray(bigwa.astype(BF)),
            bigwb=bigwb,
            constf=np.ascontiguousarray(constf),
            xb2=np.ascontiguousarray(xb2.astype(BF)),
        ))

    res = run_bass_kernel_spmd(_NC, in_maps, list(range(NCORES)))

    out = np.empty((B, C, H, W), np.float32)
    for c in range(NCORES):
        b, half = divmod(c, 2)
        h0 = half * ROWS
        out[b, :, h0:h0 + ROWS, :] = res.results[c]["out"].astype(
            np.float32).reshape(C, ROWS, W)
    return out


# revision 46
# speedup vs baseline: 1.0147x; 1.0026x over previous
"""Trainium2 Bass kernel for nn_Decoder_22196390985918 (SPADE-style decoder).

Sharding: 8 cores = (batch b in 0..3) x (H-half in 0..1). Each core computes
out[b, :, h0:h0+64, :] for h0 = 64*(core%2).

v2 restructure vs baseline:
- All weights host-side pre-transposed into final lhsT layouts, blend
  (sigmoid) factors folded in on host, cast to bf16, packed into 2 big
  DMA blobs + 1 small fp32 const blob -> ~6 input DMAs total instead of ~60.
- x loaded once as bf16 [128, 8192]: partitions 0-63 = own 64 rows
  (channel-major, reused by the epilogue), 64-127 = other half (stats only).
  Instance-norm stats via one multi-chunk bn_stats; halves combined with
  plain [64]-partition tensor ops (no transposes / DMAs).
- Region-priority mask fused: sel = (cnt == 0) * seg as one DVE
  scalar_tensor_tensor reading cnt straight from PSUM.
- sel45 partition order (j, ty, tx) so the G table rearrange is ONE
  SBUF->SBUF DMA. u5 = kron(tril, eye(9)) accordingly.
- Epilogue: out = (x - mu) * A + B with A = (psum_g + bias_g)*rstd via one
  ACT op, B = psum_b + bias_b via another; no PE broadcast matmul.
- Main conv loop starts as soon as selG is ready (~20us) and overlaps all
  remaining DMA.
"""
import numpy as np
import ml_dtypes

import concourse.bacc as bacc
import concourse.bass as bass
import concourse.mybir as mybir
import concourse.tile as tile
from concourse.bass_utils import run_bass_kernel_spmd

dt = mybir.dt
F32 = dt.float32
BF16 = dt.bfloat16
AF = mybir.ActivationFunctionType
ALU = mybir.AluOpType
BF = ml_dtypes.bfloat16

B, C, H, W, F, L, NH = 4, 64, 128, 128, 5, 512, 128
GW = 130                    # padded grid width  (image col = grid col - 1)
SR = 66                     # seg/sel/actv grid rows (image row = h0 - 1 + r)
MR = 68                     # mask grid rows (image row = h0 - 2 + r)
SEG_N = SR * GW             # 8580
MASK_N = MR * GW            # 8840
GLS = SEG_N + 48            # seg grid line length (incl. u5 tail)
GLM = MASK_N                # mask grid line length
ROWS = 64                   # output rows per core
NCH = 16                    # main conv chunks (4 rows x 128 cols, N=512)
ACH = 22                    # shared conv chunks (3 rows x 128 cols, N=384)
NCORES = 8

# bigwa layout (per-partition elem offsets): fcw [5*4*512] then codes [5*512]
OFF_FCW = 0
OFF_CODES = 5 * 4 * 512     # 10240
BWA = OFF_CODES + 5 * 512   # 12800
# bigwb layout: wct [4*9*128] then spT [9*128] then sswT [128]
OFF_WCT = 0
OFF_SPT = 4 * 9 * 128       # 4608
OFF_SSW = OFF_SPT + 9 * 128  # 5760
BWB = OFF_SSW + 128         # 5888
# constf layout (fp32): 0 biasg, 1 ssb, 2-3 hal, 4-23 fcbt, 24 biasb64
CF = 25
# u5 lives in the grids blob tail on partitions 0..44
OFF_U5 = SEG_N              # 8580..8625


def _build_nc():
    nc = bacc.Bacc()

    gseg_d = nc.dram_tensor("gseg", [46, SEG_N], BF16, kind="ExternalInput")
    gmask_d = nc.dram_tensor("gmask", [27, MASK_N], BF16,
                             kind="ExternalInput")
    u5_d = nc.dram_tensor("u5d", [45, 48], BF16, kind="ExternalInput")
    brow_d = nc.dram_tensor("brow", [1, 128], BF16, kind="ExternalInput")
    spt8_d = nc.dram_tensor("spt8", [128, 1152], dt.float8e4,
                            kind="ExternalInput")
    bigwa_d = nc.dram_tensor("bigwa", [128, BWA], BF16, kind="ExternalInput")
    bigwb_d = nc.dram_tensor("bigwb", [128, BWB], BF16, kind="ExternalInput")
    constf_d = nc.dram_tensor("constf", [128, CF], F32, kind="ExternalInput")
    xb2_d = nc.dram_tensor("xb2", [128, 8192], BF16, kind="ExternalInput")
    out_d = nc.dram_tensor("out", [C, 4, 4 * 512], BF16, kind="ExternalOutput")

    with tile.TileContext(nc) as tc:
        with (
            tc.tile_pool(name="const", bufs=1) as cst,
            tc.tile_pool(name="mus", bufs=2) as musp,
            tc.tile_pool(name="gb", bufs=3) as gbp,
            tc.tile_pool(name="ep", bufs=3) as epp,
            tc.tile_pool(name="ot", bufs=2) as otp,
            tc.tile_pool(name="pmain", bufs=4, space="PSUM") as pmain,
            tc.tile_pool(name="paux", bufs=2, space="PSUM") as paux,
            tc.tile_pool(name="pg", bufs=2, space="PSUM") as pgp,
        ):
            # ---- input DMAs ---------------------------------------------
            segc = cst.tile([16, SEG_N + 2 * GW + 4], BF16)
            nc.sync.dma_start(out=segc[:], in_=segc_d[:])
            maskc = cst.tile([9, MASK_N + 2 * GW + 4], BF16)
            nc.sync.dma_start(out=maskc[:], in_=maskc_d[:])
            u5t = cst.tile([45, 48], BF16)
            nc.sync.dma_start(out=u5t[:], in_=u5_d[:])
            spt8 = cst.tile([128, 4, 2, 128], dt.float8e4)
            spt8b = cst.tile([128, 128], dt.float8e4)
            constf = cst.tile([128, CF], F32)
            nc.gpsimd.dma_start(out=constf[:], in_=constf_d[:])
            bigwa = cst.tile([128, BWA], BF16)
            nc.gpsimd.dma_start(out=bigwa[:], in_=bigwa_d[:])
            bigwb = cst.tile([128, BWB], BF16)
            nc.sync.dma_start(out=bigwb[:], in_=bigwb_d[:])
            xb2 = cst.tile([128, 8192], BF16)
            nc.gpsimd.dma_start(out=xb2[:], in_=xb2_d[:])

            # on-chip 9-shift replication (vector DMA queue, SBUF->SBUF)
            gseg = cst.tile([46, SEG_N], BF16)
            sca = segc[:]
            nc.vector.dma_start(
                out=gseg[0:45, :],
                in_=bass.AP(tensor=sca.tensor, offset=sca.offset,
                            ap=[sca.ap[0][:1] + [5], [GW, 3], [1, 3],
                                [1, SEG_N]]))
            nc.vector.dma_start(out=gseg[45:46, :], in_=segc[5:6, 0:SEG_N])
            gmask = cst.tile([27, MASK_N], BF16)
            mca = maskc[:]
            nc.vector.dma_start(
                out=gmask[:],
                in_=bass.AP(tensor=mca.tensor, offset=mca.offset,
                            ap=[mca.ap[0][:1] + [3], [GW, 3], [1, 3],
                                [1, MASK_N]]))

            sel45 = gseg[0:45, 0:SEG_N]
            u5 = u5t[:, 0:45]
            mask27 = gmask[:, 0:MASK_N]
            epst = cst.tile([128, 1], F32)
            nc.gpsimd.memset(epst[:], 1e-5)
            rstdall = cst.tile([128, 1], F32)
            nc.gpsimd.memset(rstdall[64:128, :], 1.0 / 64.0)
            zt = cst.tile([128, 1], F32)
            nc.gpsimd.memset(zt[:], 0.0)
            biasg = constf[:, 0:1]
            ssb = constf[:, 1:2]
            hal = constf[:, 2:4]
            fcbt = constf[:, 4:24].rearrange("p (k j) -> p k j", k=4)
            biasb64 = constf[0:64, 24:25]

            # ---- region masks: cnt (PE) -> sel = (cnt==0)*seg (DVE) -----
            segchunks = []
            off = 0
            while off < SEG_N:
                n = min(512, SEG_N - off)
                segchunks.append((off, n))
                off += n
            for off, n in segchunks:
                pc = paux.tile([45, 512], F32, tag="aux")
                nc.tensor.matmul(pc[:, 0:n], u5, sel45[:, off:off + n],
                                 start=True, stop=True)
                nc.scalar.activation(sel45[:, off:off + n], pc[:, 0:n],
                                     AF.Relu, bias=zt[0:45, :])

            # ---- shared conv (mask 3 -> NH) + actv assembly -------------
            actv = cst.tile([NH, SR, GW], dt.float8e4)
            bord = actv[:, :, 0:1]
            nc.gpsimd.memset(
                bass.AP(tensor=bord.tensor, offset=bord.offset,
                        ap=[bord.ap[0], [GW, SR], [GW - 1, 2]]), 0.0)
            sswT = bigwb[0:27, OFF_SSW:OFF_SSW + 128]
            m3 = mask27.rearrange("p (r c) -> p r c", c=GW)
            for a in range(ACH):
                r = 3 * a
                psh = paux.tile([NH, 3, 128], F32, tag="aux")
                nc.tensor.matmul(psh[:], sswT, m3[:, r:r + 3, 0:128],
                                 start=True, stop=True)
                nc.scalar.activation(actv[:, r:r + 3, 1:129], psh[:], AF.Relu,
                                     bias=ssb, scale=1.0)
            nc.scalar.activation(actv[:, 0, :], actv[:, 0, :], AF.Identity,
                                  bias=zt[:], scale=hal[:, 0:1])
            nc.scalar.activation(actv[:, SR - 1, :], actv[:, SR - 1, :],
                                 AF.Identity, bias=zt[:], scale=hal[:, 1:2])

            # ---- mu path: z = fcb + sum_l fcw*codes (DVE TTR), relu -----
            z_sb = cst.tile([128, 4, F], F32)
            for j in range(F):
                cview = bigwa[:, OFF_CODES + j * L:OFF_CODES + (j + 1) * L]
                for kb in range(4):
                    fview = bigwa[:, OFF_FCW + (j * 4 + kb) * L:
                                  OFF_FCW + (j * 4 + kb + 1) * L]
                    mus = musp.tile([128, L], BF16, tag="mus")
                    nc.vector.scalar_tensor_tensor(
                        out=mus[:], in0=fview, scalar=1.0, in1=cview,
                        op0=ALU.mult, op1=ALU.mult,
                        accum_out=z_sb[:, kb, j:j + 1])
            nc.vector.tensor_add(z_sb[:], z_sb[:], fcbt[:])
            muT = cst.tile([128, 4, F], BF16)
            nc.scalar.activation(muT[:], z_sb[:], AF.Relu, bias=zt[:])

            # ---- G tables -> selG (one rearrange DMA) -------------------
            gstage = cst.tile([F, 9, 128], BF16)
            for g in range(3):
                gps = pgp.tile([F, 3, 128], F32, tag="gps")
                for kb in range(4):
                    wview = bigwb[:, OFF_WCT + kb * 1152 + g * 384:
                                  OFF_WCT + kb * 1152 + (g + 1) * 384]
                    nc.tensor.matmul(gps[:], muT[:, kb, :],
                                     wview.rearrange("p (t c) -> p t c", t=3),
                                     start=(kb == 0), stop=(kb == 3))
                nc.scalar.activation(gstage[:, 3 * g:3 * g + 3, :], gps[:], AF.Copy)
            selG = cst.tile([46, 128], BF16)
            for ty in range(3):
                nc.scalar.dma_start(out=selG[15 * ty:15 * ty + 15, :],
                                    in_=gstage[:, 3 * ty:3 * ty + 3, :])
            nc.scalar.dma_start(out=selG[45:46, :], in_=brow_d[:])

            # ---- instance-norm stats ------------------------------------
            st = cst.tile([128, 16, 6], F32)
            for q in range(16):
                nc.vector.bn_stats(out=st[:, q, :],
                                   in_=xb2[:, q * 512:(q + 1) * 512])
            mv = cst.tile([128, 2], F32)
            nc.vector.bn_aggr(out=mv[:], in_=st[:])
            mvm = cst.tile([128, 2], F32)
            nc.vector.tensor_copy(mvm[:, 0:1], mv[:, 0:1])
            nc.vector.scalar_tensor_tensor(
                out=mvm[:, 1:2], in0=mv[:, 0:1], scalar=mv[:, 0:1], in1=mv[:, 1:2],
                op0=ALU.mult, op1=ALU.add)
            oth = cst.tile([64, 2], F32)
            nc.vector.stream_shuffle(oth[:], mvm[64:128, :], list(range(32)))
            mus_ = cst.tile([64, 1], F32)
            nc.vector.tensor_add(mus_[:], mvm[0:64, 0:1], oth[:, 0:1])
            m2s = cst.tile([64, 1], F32)
            nc.vector.tensor_add(m2s[:], mvm[0:64, 1:2], oth[:, 1:2])
            muc = cst.tile([64, 1], F32)
            nc.vector.tensor_scalar_mul(muc[:], mus_[:], 0.5)
            mu2 = cst.tile([64, 1], F32)
            nc.vector.tensor_mul(mu2[:], muc[:], muc[:])
            varc = cst.tile([64, 1], F32)
            nc.vector.scalar_tensor_tensor(
                out=varc[:], in0=m2s[:], scalar=0.5, in1=mu2[:],
                op0=ALU.mult, op1=ALU.subtract)
            ve = cst.tile([64, 1], F32)
            nc.vector.tensor_scalar_add(ve[:], varc[:], 1e-5)
            ri = cst.tile([64, 1], dt.int32)
            nc.vector.tensor_scalar(ri[:], ve[:].bitcast(dt.int32),
                                    1, None, op0=ALU.arith_shift_right)
            nc.vector.tensor_scalar(ri[:], ri[:], 0x5f3759df, -1,
                                    op0=ALU.subtract, op1=ALU.mult)
            r0 = cst.tile([64, 1], F32)
            nc.vector.tensor_copy(r0[:], ri[:].bitcast(F32))
            rstd = cst.tile([64, 1], F32)
            ra = cst.tile([64, 1], F32)
            rb = cst.tile([64, 1], F32)
            for it in range(2):
                nc.vector.tensor_mul(ra[:], r0[:], r0[:])
                nc.vector.tensor_mul(rb[:], ra[:], ve[:])
                nc.vector.tensor_scalar(rb[:], rb[:], -0.5, 1.5,
                                        op0=ALU.mult, op1=ALU.add)
                nc.vector.tensor_mul(r0[:], r0[:], rb[:])
            nc.vector.tensor_copy(rstd[:], r0[:])
            rstd64 = cst.tile([64, 1], F32)
            nc.vector.tensor_scalar_mul(rstd64[:], rstd[:], 1.0 / 64.0)
            nc.vector.tensor_copy(rstdall[0:64, :], rstd64[:])

            # ---- main conv + epilogue (epilogue one chunk behind) -------
            s3 = gseg[:].rearrange("p (r c) -> p r c", c=GW)
            a3 = actv[:]
            pms = {}
            ots = {}

            aoff = [ty * GW + tx for ty in range(3) for tx in range(3)]
            a8 = actv[:]

            def conv_chunk(i):
                pm = pmain.tile([128, 4, 128], F32, tag="pm", name=f"pm_{i}")
                pms[i] = pm
                for P in range(4):
                    o0, o1 = aoff[2 * P], aoff[2 * P + 1]
                    rhs = bass.AP(
                        tensor=a8.tensor,
                        offset=a8.offset + 4 * i * GW + o0,
                        ap=[a8.ap[0], [o1 - o0, 2], [GW, 4], [1, 128]])
                    nc.tensor.matmul(pm[:], spt8[:, P, :, :], rhs,
                                     start=(P == 0), stop=False,
                                     perf_mode=mybir.MatmulPerfMode.DoubleRow,
                                     skip_group_check=True)
                rhs8 = bass.AP(tensor=a8.tensor,
                               offset=a8.offset + 4 * i * GW + aoff[8],
                               ap=[a8.ap[0], [GW, 4], [1, 128]])
                nc.tensor.matmul(pm[:], spt8b[:], rhs8, start=False, stop=False,
                                 skip_group_check=True)
                nc.tensor.matmul(pm[:], selG[:], s3[:, 4 * i:4 * i + 4, 0:128],
                                 start=False, stop=True, skip_group_check=True)

            def epi_chunk(i):
                pm = pms.pop(i)
                if i % 4 == 0:
                    ots[i // 4] = otp.tile([64, 4, 4, 128], BF16, tag="ot",
                                           name=f"ot_{i // 4}")
                ot = ots[i // 4]
                gt = gbp.tile([128, 4, 128], F32, tag="gb")
                nc.scalar.activation(gt[:], pm[:], AF.Identity,
                                     bias=zt[:], scale=rstdall[:])
                ga = gt[0:64, :, :]
                bbm = epp.tile([64, 4, 128], F32, tag="ep")
                nc.vector.stream_shuffle(bbm[:], gt[64:128, :, :], list(range(32)))
                xa = epp.tile([64, 4, 128], F32, tag="ep")
                nc.vector.scalar_tensor_tensor(
                    out=xa[:], in0=xb2[0:64, i * 512:(i + 1) * 512].rearrange(
                        "p (r w) -> p r w", r=4),
                    scalar=muc[:], in1=ga, op0=ALU.subtract,
                    op1=ALU.mult)
                nc.gpsimd.tensor_add(ot[:, i % 4, :, :], xa[:], bbm[:])
                if i % 4 == 3:
                    q = i // 4
                    nc.sync.dma_start(
                        out=out_d[:, q, :],
                        in_=ots.pop(q)[:].rearrange("c k r w -> c (k r w)"))

            conv_chunk(0)
            for i in range(1, NCH):
                conv_chunk(i)
                epi_chunk(i - 1)
            epi_chunk(NCH - 1)

    nc.finalize()
    return nc


_NC = None


def kernel(**inputs):
    global _NC
    x = np.asarray(inputs["x"], dtype=np.float32)
    segmap = np.asarray(inputs["segmap"], dtype=np.float32)
    codes_vector = np.asarray(inputs["codes_vector"], dtype=np.float32)
    mask = np.asarray(inputs["mask"], dtype=np.float32)
    fc_w = np.asarray(inputs["fc_w"], dtype=np.float32)
    fc_b = np.asarray(inputs["fc_b"], dtype=np.float32)
    cgw = np.asarray(inputs["conv_gamma_w"], dtype=np.float32)
    cgb = np.asarray(inputs["conv_gamma_b"], dtype=np.float32)
    cbw = np.asarray(inputs["conv_beta_w"], dtype=np.float32)
    cbb = np.asarray(inputs["conv_beta_b"], dtype=np.float32)
    ssw = np.asarray(inputs["spade_shared_w"], dtype=np.float32)
    ssb = np.asarray(inputs["spade_shared_b"], dtype=np.float32)
    sgw = np.asarray(inputs["spade_gamma_w"], dtype=np.float32)
    sgb = np.asarray(inputs["spade_gamma_b"], dtype=np.float32)
    sbw = np.asarray(inputs["spade_beta_w"], dtype=np.float32)
    sbb = np.asarray(inputs["spade_beta_b"], dtype=np.float32)
    bg = float(np.asarray(inputs["blending_gamma"]).reshape(-1)[0])
    bb_ = float(np.asarray(inputs["blending_beta"]).reshape(-1)[0])

    if _NC is None:
        _NC = _build_nc()

    ga = 1.0 / (1.0 + np.exp(-bg))
    ba = 1.0 / (1.0 + np.exp(-bb_))

    # bigwb: wct | spT | sswT  (shared across cores)
    bigwb = np.zeros((128, BWB), np.float32)
    # wct[p, kb*1152 + (3ty+tx)*128 + cc] = blend * conv_w[cc, kb*128+p, ty, tx]
    cw = np.concatenate([cgw * ga, cbw * ba], axis=0) * 64.0   # [128, 512, 3, 3]
    wct = cw.reshape(128, 4, 128, 9).transpose(2, 1, 3, 0)     # [p, kb, t, cc]
    bigwb[:, OFF_WCT:OFF_SPT] = wct.reshape(128, 4608)
    sw = np.concatenate([sgw * (1 - ga), sbw * (1 - ba)], axis=0)  # [128, NH, 3, 3]
    spT = sw.reshape(128, 128, 9).transpose(1, 2, 0) * 64.0    # [nh, t, cc]
    spt8h = np.zeros((128, 1152), np.float32)
    spt8h[:, 0:1024] = spT[:, 0:8, :].reshape(128, 1024)
    spt8h[:, 1024:1152] = spT[:, 8, :]
    spt8h = np.ascontiguousarray(spt8h.astype(ml_dtypes.float8_e4m3))
    # sswT[9ty+3c+tx, nh] = ssw[nh, c, ty, tx] (on-chip mask replicate order)
    sswT = ssw.transpose(2, 1, 3, 0).reshape(27, 128)
    bigwb[0:27, OFF_SSW:OFF_SSW + 128] = sswT
    bigwb = bigwb.astype(BF)

    # constf: biasg | ssb | hal | fcbt (hal per-core, rest shared)
    constf_base = np.zeros((128, CF), np.float32)
    constf_base[0:64, 0] = ga * cgb + (1 - ga) * sgb + 1.0
    constf_base[64:128, 0] = ba * cbb + (1 - ba) * sbb
    constf_base[:, 1] = ssb
    constf_base[0:64, 24] = ba * cbb + (1 - ba) * sbb
    # fcbt[p, 4 + kb*5 + j] = fc_b[j, kb*128+p]
    constf_base[:, 4:24] = fc_b.T.reshape(4, 128, F).transpose(1, 0, 2).reshape(128, 20)

    # u5[9j'+t', 9j+t] = (j' > j) * (t'==t)
    u5 = (np.eye(45, dtype=np.float32)
          - np.kron(np.eye(3, dtype=np.float32),
                    np.kron(np.tril(np.ones((F, F), np.float32), -1),
                            np.eye(3, dtype=np.float32))))
    brow = np.zeros((1, 128), np.float32)
    brow[0, 0:64] = (ga * cgb + (1 - ga) * sgb + 1.0) * 64.0
    brow[0, 64:128] = (ba * cbb + (1 - ba) * sbb) * 64.0
    brow = brow.astype(BF)
    u5h = np.zeros((45, 48), np.float32)
    u5h[:, 0:45] = u5
    u5h = np.ascontiguousarray(u5h.astype(BF))

    in_maps = []
    for c in range(NCORES):
        b, half = divmod(c, 2)
        h0 = half * ROWS

        # bigwa: fcw (k-partition, l-free) | codes broadcast (per-batch)
        bigwa = np.zeros((128, BWA), np.float32)
        # fcw_sec[p, (j*4+kb)*512 + l] = fc_w[j, kb*128+p, l]
        bigwa[:, OFF_FCW:OFF_CODES] = (
            fc_w.reshape(F, 4, 128, L).transpose(2, 0, 1, 3).reshape(128, 10240))
        bigwa[:, OFF_CODES:BWA] = np.broadcast_to(
            codes_vector[b].reshape(1, F * L), (128, F * L))

        gsegh = np.ones((46, SEG_N), np.float32)
        gmaskh = np.zeros((27, MASK_N), np.float32)
        segp = np.zeros((F, SR + 2, GW + 2), np.float32)
        r_lo, r_hi = h0 - 1, h0 + ROWS + 1
        s_lo, s_hi = max(r_lo, 0), min(r_hi, H)
        segp[:, s_lo - r_lo:s_hi - r_lo, 1:129] = segmap[b, :, s_lo:s_hi, :]
        for ty in range(3):
            for j in range(F):
                for tx in range(3):
                    gsegh[15 * ty + 3 * j + tx, :] = (
                        segp[j, ty:ty + SR, tx:tx + GW].reshape(-1))
        maskp = np.zeros((3, MR + 2, GW + 2), np.float32)
        m_lo, m_hi = h0 - 2, h0 + ROWS + 2
        ms_lo, ms_hi = max(m_lo, 0), min(m_hi, H)
        maskp[:, ms_lo - m_lo:ms_hi - m_lo, 1:129] = mask[b, :, ms_lo:ms_hi, :]
        for ty in range(3):
            for cc in range(3):
                for tx in range(3):
                    gmaskh[9 * ty + 3 * cc + tx, :] = (
                        maskp[cc, ty:ty + MR, tx:tx + GW].reshape(-1))

        constf = constf_base.copy()
        constf[:, 2] = 0.0 if h0 == 0 else 1.0
        constf[:, 3] = 0.0 if h0 + ROWS == H else 1.0

        xb2 = np.concatenate([
            x[b, :, h0:h0 + ROWS, :].reshape(C, 8192),
            x[b, :, ROWS - h0:ROWS - h0 + ROWS, :].reshape(C, 8192)], axis=0)

        in_maps.append(dict(
            brow=brow,
            spt8=spt8h,
            u5d=u5h,
            gseg=np.ascontiguousarray(gsegh.astype(BF)),
            gmask=np.ascontiguousarray(gmaskh.astype(BF)),
            bigwa=np.ascontiguousarray(bigwa.astype(BF)),
            bigwb=bigwb,
            constf=np.ascontiguousarray(constf),
            xb2=np.ascontiguousarray(xb2.astype(BF)),
        ))

    res = run_bass_kernel_spmd(_NC, in_maps, list(range(NCORES)))

    out = np.empty((B, C, H, W), np.float32)
    for c in range(NCORES):
        b, half = divmod(c, 2)
        h0 = half * ROWS
        out[b, :, h0:h0 + ROWS, :] = res.results[c]["out"].astype(
            np.float32).reshape(C, ROWS, W)
    return out


# revision 47
# speedup vs baseline: 1.0668x; 1.0513x over previous
"""Trainium2 Bass kernel for nn_Decoder_22196390985918 (SPADE-style decoder).

Sharding: 8 cores = (batch b in 0..3) x (H-half in 0..1). Each core computes
out[b, :, h0:h0+64, :] for h0 = 64*(core%2).

v2 restructure vs baseline:
- All weights host-side pre-transposed into final lhsT layouts, blend
  (sigmoid) factors folded in on host, cast to bf16, packed into 2 big
  DMA blobs + 1 small fp32 const blob -> ~6 input DMAs total instead of ~60.
- x loaded once as bf16 [128, 8192]: partitions 0-63 = own 64 rows
  (channel-major, reused by the epilogue), 64-127 = other half (stats only).
  Instance-norm stats via one multi-chunk bn_stats; halves combined with
  plain [64]-partition tensor ops (no transposes / DMAs).
- Region-priority mask fused: sel = (cnt == 0) * seg as one DVE
  scalar_tensor_tensor reading cnt straight from PSUM.
- sel45 partition order (j, ty, tx) so the G table rearrange is ONE
  SBUF->SBUF DMA. u5 = kron(tril, eye(9)) accordingly.
- Epilogue: out = (x - mu) * A + B with A = (psum_g + bias_g)*rstd via one
  ACT op, B = psum_b + bias_b via another; no PE broadcast matmul.
- Main conv loop starts as soon as selG is ready (~20us) and overlaps all
  remaining DMA.
"""
import numpy as np
import ml_dtypes

import concourse.bacc as bacc
import concourse.bass as bass
import concourse.mybir as mybir
import concourse.tile as tile
from concourse.bass_utils import run_bass_kernel_spmd

dt = mybir.dt
F32 = dt.float32
BF16 = dt.bfloat16
AF = mybir.ActivationFunctionType
ALU = mybir.AluOpType
BF = ml_dtypes.bfloat16

B, C, H, W, F, L, NH = 4, 64, 128, 128, 5, 512, 128
GW = 130                    # padded grid width  (image col = grid col - 1)
SR = 66                     # seg/sel/actv grid rows (image row = h0 - 1 + r)
MR = 68                     # mask grid rows (image row = h0 - 2 + r)
SEG_N = SR * GW             # 8580
MASK_N = MR * GW            # 8840
GLS = SEG_N + 48            # seg grid line length (incl. u5 tail)
GLM = MASK_N                # mask grid line length
ROWS = 64                   # output rows per core
NCH = 16                    # main conv chunks (4 rows x 128 cols, N=512)
ACH = 22                    # shared conv chunks (3 rows x 128 cols, N=384)
NCORES = 8

# bigwa layout (per-partition elem offsets): fcw [5*4*512] then codes [5*512]
OFF_FCW = 0
OFF_CODES = 5 * 4 * 512     # 10240
BWA = OFF_CODES + 5 * 512   # 12800
# bigwb layout: wct [4*9*128] then spT [9*128] then sswT [128]
OFF_WCT = 0
OFF_SPT = 4 * 9 * 128       # 4608
OFF_SSW = OFF_SPT + 9 * 128  # 5760
BWB = OFF_SSW + 128         # 5888
# constf layout (fp32): 0 biasg, 1 ssb, 2-3 hal, 4-23 fcbt, 24 biasb64
CF = 25
# u5 lives in the grids blob tail on partitions 0..44
OFF_U5 = SEG_N              # 8580..8625


def _build_nc():
    nc = bacc.Bacc()

    gseg_d = nc.dram_tensor("gseg", [46, SEG_N], BF16, kind="ExternalInput")
    gmask_d = nc.dram_tensor("gmask", [27, MASK_N], BF16,
                             kind="ExternalInput")
    u5_d = nc.dram_tensor("u5d", [45, 48], BF16, kind="ExternalInput")
    brow_d = nc.dram_tensor("brow", [1, 128], BF16, kind="ExternalInput")
    spt8_d = nc.dram_tensor("spt8", [128, 1152], dt.float8e4,
                            kind="ExternalInput")
    bigwa_d = nc.dram_tensor("bigwa", [128, BWA], BF16, kind="ExternalInput")
    bigwb_d = nc.dram_tensor("bigwb", [128, BWB], BF16, kind="ExternalInput")
    constf_d = nc.dram_tensor("constf", [128, CF], F32, kind="ExternalInput")
    xb2_d = nc.dram_tensor("xb2", [128, 8192], BF16, kind="ExternalInput")
    out_d = nc.dram_tensor("out", [C, 4, 4 * 512], BF16, kind="ExternalOutput")

    with tile.TileContext(nc) as tc:
        with (
            tc.tile_pool(name="const", bufs=1) as cst,
            tc.tile_pool(name="mus", bufs=2) as musp,
            tc.tile_pool(name="gb", bufs=3) as gbp,
            tc.tile_pool(name="ep", bufs=3) as epp,
            tc.tile_pool(name="ot", bufs=2) as otp,
            tc.tile_pool(name="pmain", bufs=3, space="PSUM") as pmain,
            tc.tile_pool(name="paux", bufs=3, space="PSUM") as paux,
            tc.tile_pool(name="pg", bufs=2, space="PSUM") as pgp,
        ):
            # ---- input DMAs ---------------------------------------------
            segc = cst.tile([16, SEG_N + 2 * GW + 4], BF16)
            nc.sync.dma_start(out=segc[:], in_=segc_d[:])
            maskc = cst.tile([9, MASK_N + 2 * GW + 4], BF16)
            nc.sync.dma_start(out=maskc[:], in_=maskc_d[:])
            u5t = cst.tile([45, 48], BF16)
            nc.sync.dma_start(out=u5t[:], in_=u5_d[:])
            spt8 = cst.tile([128, 4, 2, 128], dt.float8e4)
            spt8b = cst.tile([128, 128], dt.float8e4)
            constf = cst.tile([128, CF], F32)
            nc.gpsimd.dma_start(out=constf[:], in_=constf_d[:])
            bigwa = cst.tile([128, BWA], BF16)
            nc.gpsimd.dma_start(out=bigwa[:], in_=bigwa_d[:])
            bigwb = cst.tile([128, BWB], BF16)
            nc.sync.dma_start(out=bigwb[:], in_=bigwb_d[:])
            xb2 = cst.tile([128, 8192], BF16)
            nc.gpsimd.dma_start(out=xb2[:], in_=xb2_d[:])

            # on-chip 9-shift replication (vector DMA queue, SBUF->SBUF)
            gseg = cst.tile([46, SEG_N], BF16)
            sca = segc[:]
            nc.vector.dma_start(
                out=gseg[0:45, :],
                in_=bass.AP(tensor=sca.tensor, offset=sca.offset,
                            ap=[sca.ap[0][:1] + [5], [GW, 3], [1, 3],
                                [1, SEG_N]]))
            nc.vector.dma_start(out=gseg[45:46, :], in_=segc[5:6, 0:SEG_N])
            gmask = cst.tile([27, MASK_N], BF16)
            mca = maskc[:]
            nc.vector.dma_start(
                out=gmask[:],
                in_=bass.AP(tensor=mca.tensor, offset=mca.offset,
                            ap=[mca.ap[0][:1] + [3], [GW, 3], [1, 3],
                                [1, MASK_N]]))

            sel45 = gseg[0:45, 0:SEG_N]
            u5 = u5t[:, 0:45]
            mask27 = gmask[:, 0:MASK_N]
            epst = cst.tile([128, 1], F32)
            nc.gpsimd.memset(epst[:], 1e-5)
            rstdall = cst.tile([128, 1], F32)
            nc.gpsimd.memset(rstdall[64:128, :], 1.0 / 64.0)
            zt = cst.tile([128, 1], F32)
            nc.gpsimd.memset(zt[:], 0.0)
            biasg = constf[:, 0:1]
            ssb = constf[:, 1:2]
            hal = constf[:, 2:4]
            fcbt = constf[:, 4:24].rearrange("p (k j) -> p k j", k=4)
            biasb64 = constf[0:64, 24:25]

            # ---- region masks: cnt (PE) -> sel = (cnt==0)*seg (DVE) -----
            segchunks = []
            off = 0
            while off < SEG_N:
                n = min(512, SEG_N - off)
                segchunks.append((off, n))
                off += n
            for ci, (off, n) in enumerate(segchunks):
                pc = paux.tile([45, 512], F32, tag="aux")
                nc.tensor.matmul(pc[:, 0:n], u5, sel45[:, off:off + n],
                                 start=True, stop=True)
                if ci % 3 == 2:
                    nc.vector.tensor_scalar_max(sel45[:, off:off + n],
                                                pc[:, 0:n], 0.0)
                else:
                    nc.scalar.activation(sel45[:, off:off + n], pc[:, 0:n],
                                         AF.Relu, bias=zt[0:45, :])

            # ---- shared conv (mask 3 -> NH) + actv assembly -------------
            actv = cst.tile([NH, SR, GW], dt.float8e4)
            bord = actv[:, :, 0:1]
            nc.gpsimd.memset(
                bass.AP(tensor=bord.tensor, offset=bord.offset,
                        ap=[bord.ap[0], [GW, SR], [GW - 1, 2]]), 0.0)
            sswT = bigwb[0:27, OFF_SSW:OFF_SSW + 128]
            m3 = mask27.rearrange("p (r c) -> p r c", c=GW)
            for a in range(ACH):
                r = 3 * a
                psh = paux.tile([NH, 3, 128], F32, tag="aux")
                nc.tensor.matmul(psh[:], sswT, m3[:, r:r + 3, 0:128],
                                 start=True, stop=True)
                if a % 3 == 2:
                    nc.vector.tensor_scalar(actv[:, r:r + 3, 1:129], psh[:],
                                            ssb, 0.0, op0=ALU.add, op1=ALU.max)
                else:
                    nc.scalar.activation(actv[:, r:r + 3, 1:129], psh[:],
                                         AF.Relu, bias=ssb, scale=1.0)
            nc.scalar.activation(actv[:, 0, :], actv[:, 0, :], AF.Identity,
                                  bias=zt[:], scale=hal[:, 0:1])
            nc.scalar.activation(actv[:, SR - 1, :], actv[:, SR - 1, :],
                                 AF.Identity, bias=zt[:], scale=hal[:, 1:2])

            # ---- mu path: z = fcb + sum_l fcw*codes (DVE TTR), relu -----
            z_sb = cst.tile([128, 4, F], F32)
            for j in range(F):
                cview = bigwa[:, OFF_CODES + j * L:OFF_CODES + (j + 1) * L]
                for kb in range(4):
                    fview = bigwa[:, OFF_FCW + (j * 4 + kb) * L:
                                  OFF_FCW + (j * 4 + kb + 1) * L]
                    mus = musp.tile([128, L], BF16, tag="mus")
                    nc.vector.scalar_tensor_tensor(
                        out=mus[:], in0=fview, scalar=1.0, in1=cview,
                        op0=ALU.mult, op1=ALU.mult,
                        accum_out=z_sb[:, kb, j:j + 1])
            nc.vector.tensor_add(z_sb[:], z_sb[:], fcbt[:])
            muT = cst.tile([128, 4, F], BF16)
            nc.vector.tensor_scalar_max(muT[:], z_sb[:], 0.0)

            # ---- G tables -> selG (one rearrange DMA) -------------------
            gstage = cst.tile([F, 9, 128], BF16)
            for g in range(3):
                gps = pgp.tile([F, 3, 128], F32, tag="gps")
                for kb in range(4):
                    wview = bigwb[:, OFF_WCT + kb * 1152 + g * 384:
                                  OFF_WCT + kb * 1152 + (g + 1) * 384]
                    nc.tensor.matmul(gps[:], muT[:, kb, :],
                                     wview.rearrange("p (t c) -> p t c", t=3),
                                     start=(kb == 0), stop=(kb == 3))
                nc.scalar.activation(gstage[:, 3 * g:3 * g + 3, :], gps[:], AF.Copy)
            selG = cst.tile([46, 128], BF16)
            for ty in range(3):
                nc.scalar.dma_start(out=selG[15 * ty:15 * ty + 15, :],
                                    in_=gstage[:, 3 * ty:3 * ty + 3, :])
            nc.scalar.dma_start(out=selG[45:46, :], in_=brow_d[:])

            # ---- instance-norm stats ------------------------------------
            st = cst.tile([128, 16, 6], F32)
            for q in range(16):
                nc.vector.bn_stats(out=st[:, q, :],
                                   in_=xb2[:, q * 512:(q + 1) * 512])
            mv = cst.tile([128, 2], F32)
            nc.vector.bn_aggr(out=mv[:], in_=st[:])
            mvm = cst.tile([128, 2], F32)
            nc.vector.tensor_copy(mvm[:, 0:1], mv[:, 0:1])
            nc.vector.scalar_tensor_tensor(
                out=mvm[:, 1:2], in0=mv[:, 0:1], scalar=mv[:, 0:1], in1=mv[:, 1:2],
                op0=ALU.mult, op1=ALU.add)
            oth = cst.tile([64, 2], F32)
            nc.vector.stream_shuffle(oth[:], mvm[64:128, :], list(range(32)))
            mus_ = cst.tile([64, 1], F32)
            nc.vector.tensor_add(mus_[:], mvm[0:64, 0:1], oth[:, 0:1])
            m2s = cst.tile([64, 1], F32)
            nc.vector.tensor_add(m2s[:], mvm[0:64, 1:2], oth[:, 1:2])
            muc = cst.tile([64, 1], F32)
            nc.vector.tensor_scalar_mul(muc[:], mus_[:], 0.5)
            mu2 = cst.tile([64, 1], F32)
            nc.vector.tensor_mul(mu2[:], muc[:], muc[:])
            varc = cst.tile([64, 1], F32)
            nc.vector.scalar_tensor_tensor(
                out=varc[:], in0=m2s[:], scalar=0.5, in1=mu2[:],
                op0=ALU.mult, op1=ALU.subtract)
            ve = cst.tile([64, 1], F32)
            nc.vector.tensor_scalar_add(ve[:], varc[:], 1e-5)
            ri = cst.tile([64, 1], dt.int32)
            nc.vector.tensor_scalar(ri[:], ve[:].bitcast(dt.int32),
                                    1, None, op0=ALU.arith_shift_right)
            nc.vector.tensor_scalar(ri[:], ri[:], 0x5f3759df, -1,
                                    op0=ALU.subtract, op1=ALU.mult)
            r0 = cst.tile([64, 1], F32)
            nc.vector.tensor_copy(r0[:], ri[:].bitcast(F32))
            rstd = cst.tile([64, 1], F32)
            ra = cst.tile([64, 1], F32)
            rb = cst.tile([64, 1], F32)
            for it in range(2):
                nc.vector.tensor_mul(ra[:], r0[:], r0[:])
                nc.vector.tensor_mul(rb[:], ra[:], ve[:])
                nc.vector.tensor_scalar(rb[:], rb[:], -0.5, 1.5,
                                        op0=ALU.mult, op1=ALU.add)
                nc.vector.tensor_mul(r0[:], r0[:], rb[:])
            nc.vector.tensor_copy(rstd[:], r0[:])
            rstd64 = cst.tile([64, 1], F32)
            nc.vector.tensor_scalar_mul(rstd64[:], rstd[:], 1.0 / 64.0)
            nc.vector.tensor_copy(rstdall[0:64, :], rstd64[:])

            # ---- main conv + epilogue (epilogue one chunk behind) -------
            s3 = gseg[:].rearrange("p (r c) -> p r c", c=GW)
            a3 = actv[:]
            pms = {}
            ots = {}

            aoff = [ty * GW + tx for ty in range(3) for tx in range(3)]
            a8 = actv[:]

            def conv_chunk(i):
                pm = pmain.tile([128, 4, 128], F32, tag="pm", name=f"pm_{i}")
                pms[i] = pm
                for P in range(4):
                    o0, o1 = aoff[2 * P], aoff[2 * P + 1]
                    rhs = bass.AP(
                        tensor=a8.tensor,
                        offset=a8.offset + 4 * i * GW + o0,
                        ap=[a8.ap[0], [o1 - o0, 2], [GW, 4], [1, 128]])
                    nc.tensor.matmul(pm[:], spt8[:, P, :, :], rhs,
                                     start=(P == 0), stop=False,
                                     perf_mode=mybir.MatmulPerfMode.DoubleRow,
                                     skip_group_check=True)
                rhs8 = bass.AP(tensor=a8.tensor,
                               offset=a8.offset + 4 * i * GW + aoff[8],
                               ap=[a8.ap[0], [GW, 4], [1, 128]])
                nc.tensor.matmul(pm[:], spt8b[:], rhs8, start=False, stop=False,
                                 skip_group_check=True)
                nc.tensor.matmul(pm[:], selG[:], s3[:, 4 * i:4 * i + 4, 0:128],
                                 start=False, stop=True, skip_group_check=True)

            def epi_chunk(i):
                pm = pms.pop(i)
                if i % 4 == 0:
                    ots[i // 4] = otp.tile([64, 4, 4, 128], BF16, tag="ot",
                                           name=f"ot_{i // 4}")
                ot = ots[i // 4]
                gt = gbp.tile([128, 4, 128], F32, tag="gb")
                nc.scalar.activation(gt[:], pm[:], AF.Identity,
                                     bias=zt[:], scale=rstdall[:])
                ga = gt[0:64, :, :]
                bbm = epp.tile([64, 4, 128], F32, tag="ep")
                nc.vector.stream_shuffle(bbm[:], gt[64:128, :, :], list(range(32)))
                xa = epp.tile([64, 4, 128], F32, tag="ep")
                nc.vector.scalar_tensor_tensor(
                    out=xa[:], in0=xb2[0:64, i * 512:(i + 1) * 512].rearrange(
                        "p (r w) -> p r w", r=4),
                    scalar=muc[:], in1=ga, op0=ALU.subtract,
                    op1=ALU.mult)
                nc.gpsimd.tensor_add(ot[:, i % 4, :, :], xa[:], bbm[:])
                if i % 4 == 3:
                    q = i // 4
                    nc.sync.dma_start(
                        out=out_d[:, q, :],
                        in_=ots.pop(q)[:].rearrange("c k r w -> c (k r w)"))

            conv_chunk(0)
            for i in range(1, NCH):
                conv_chunk(i)
                epi_chunk(i - 1)
            epi_chunk(NCH - 1)

    nc.finalize()
    return nc


_NC = None


def kernel(**inputs):
    global _NC
    x = np.asarray(inputs["x"], dtype=np.float32)
    segmap = np.asarray(inputs["segmap"], dtype=np.float32)
    codes_vector = np.asarray(inputs["codes_vector"], dtype=np.float32)
    mask = np.asarray(inputs["mask"], dtype=np.float32)
    fc_w = np.asarray(inputs["fc_w"], dtype=np.float32)
    fc_b = np.asarray(inputs["fc_b"], dtype=np.float32)
    cgw = np.asarray(inputs["conv_gamma_w"], dtype=np.float32)
    cgb = np.asarray(inputs["conv_gamma_b"], dtype=np.float32)
    cbw = np.asarray(inputs["conv_beta_w"], dtype=np.float32)
    cbb = np.asarray(inputs["conv_beta_b"], dtype=np.float32)
    ssw = np.asarray(inputs["spade_shared_w"], dtype=np.float32)
    ssb = np.asarray(inputs["spade_shared_b"], dtype=np.float32)
    sgw = np.asarray(inputs["spade_gamma_w"], dtype=np.float32)
    sgb = np.asarray(inputs["spade_gamma_b"], dtype=np.float32)
    sbw = np.asarray(inputs["spade_beta_w"], dtype=np.float32)
    sbb = np.asarray(inputs["spade_beta_b"], dtype=np.float32)
    bg = float(np.asarray(inputs["blending_gamma"]).reshape(-1)[0])
    bb_ = float(np.asarray(inputs["blending_beta"]).reshape(-1)[0])

    if _NC is None:
        _NC = _build_nc()

    ga = 1.0 / (1.0 + np.exp(-bg))
    ba = 1.0 / (1.0 + np.exp(-bb_))

    # bigwb: wct | spT | sswT  (shared across cores)
    bigwb = np.zeros((128, BWB), np.float32)
    # wct[p, kb*1152 + (3ty+tx)*128 + cc] = blend * conv_w[cc, kb*128+p, ty, tx]
    cw = np.concatenate([cgw * ga, cbw * ba], axis=0) * 64.0   # [128, 512, 3, 3]
    wct = cw.reshape(128, 4, 128, 9).transpose(2, 1, 3, 0)     # [p, kb, t, cc]
    bigwb[:, OFF_WCT:OFF_SPT] = wct.reshape(128, 4608)
    sw = np.concatenate([sgw * (1 - ga), sbw * (1 - ba)], axis=0)  # [128, NH, 3, 3]
    spT = sw.reshape(128, 128, 9).transpose(1, 2, 0) * 64.0    # [nh, t, cc]
    spt8h = np.zeros((128, 1152), np.float32)
    spt8h[:, 0:1024] = spT[:, 0:8, :].reshape(128, 1024)
    spt8h[:, 1024:1152] = spT[:, 8, :]
    spt8h = np.ascontiguousarray(spt8h.astype(ml_dtypes.float8_e4m3))
    # sswT[9ty+3c+tx, nh] = ssw[nh, c, ty, tx] (on-chip mask replicate order)
    sswT = ssw.transpose(2, 1, 3, 0).reshape(27, 128)
    bigwb[0:27, OFF_SSW:OFF_SSW + 128] = sswT
    bigwb = bigwb.astype(BF)

    # constf: biasg | ssb | hal | fcbt (hal per-core, rest shared)
    constf_base = np.zeros((128, CF), np.float32)
    constf_base[0:64, 0] = ga * cgb + (1 - ga) * sgb + 1.0
    constf_base[64:128, 0] = ba * cbb + (1 - ba) * sbb
    constf_base[:, 1] = ssb
    constf_base[0:64, 24] = ba * cbb + (1 - ba) * sbb
    # fcbt[p, 4 + kb*5 + j] = fc_b[j, kb*128+p]
    constf_base[:, 4:24] = fc_b.T.reshape(4, 128, F).transpose(1, 0, 2).reshape(128, 20)

    # u5[9j'+t', 9j+t] = (j' > j) * (t'==t)
    u5 = (np.eye(45, dtype=np.float32)
          - np.kron(np.eye(3, dtype=np.float32),
                    np.kron(np.tril(np.ones((F, F), np.float32), -1),
                            np.eye(3, dtype=np.float32))))
    brow = np.zeros((1, 128), np.float32)
    brow[0, 0:64] = (ga * cgb + (1 - ga) * sgb + 1.0) * 64.0
    brow[0, 64:128] = (ba * cbb + (1 - ba) * sbb) * 64.0
    brow = brow.astype(BF)
    u5h = np.zeros((45, 48), np.float32)
    u5h[:, 0:45] = u5
    u5h = np.ascontiguousarray(u5h.astype(BF))

    in_maps = []
    for c in range(NCORES):
        b, half = divmod(c, 2)
        h0 = half * ROWS

        # bigwa: fcw (k-partition, l-free) | codes broadcast (per-batch)
        bigwa = np.zeros((128, BWA), np.float32)
        # fcw_sec[p, (j*4+kb)*512 + l] = fc_w[j, kb*128+p, l]
        bigwa[:, OFF_FCW:OFF_CODES] = (
            fc_w.reshape(F, 4, 128, L).transpose(2, 0, 1, 3).reshape(128, 10240))
        bigwa[:, OFF_CODES:BWA] = np.broadcast_to(
            codes_vector[b].reshape(1, F * L), (128, F * L))

        gsegh = np.ones((46, SEG_N), np.float32)
        gmaskh = np.zeros((27, MASK_N), np.float32)
        segp = np.zeros((F, SR + 2, GW + 2), np.float32)
        r_lo, r_hi = h0 - 1, h0 + ROWS + 1
        s_lo, s_hi = max(r_lo, 0), min(r_hi, H)
        segp[:, s_lo - r_lo:s_hi - r_lo, 1:129] = segmap[b, :, s_lo:s_hi, :]
        for ty in range(3):
            for j in range(F):
                for tx in range(3):
                    gsegh[15 * ty + 3 * j + tx, :] = (
                        segp[j, ty:ty + SR, tx:tx + GW].reshape(-1))
        maskp = np.zeros((3, MR + 2, GW + 2), np.float32)
        m_lo, m_hi = h0 - 2, h0 + ROWS + 2
        ms_lo, ms_hi = max(m_lo, 0), min(m_hi, H)
        maskp[:, ms_lo - m_lo:ms_hi - m_lo, 1:129] = mask[b, :, ms_lo:ms_hi, :]
        for ty in range(3):
            for cc in range(3):
                for tx in range(3):
                    gmaskh[9 * ty + 3 * cc + tx, :] = (
                        maskp[cc, ty:ty + MR, tx:tx + GW].reshape(-1))

        constf = constf_base.copy()
        constf[:, 2] = 0.0 if h0 == 0 else 1.0
        constf[:, 3] = 0.0 if h0 + ROWS == H else 1.0

        xb2 = np.concatenate([
            x[b, :, h0:h0 + ROWS, :].reshape(C, 8192),
            x[b, :, ROWS - h0:ROWS - h0 + ROWS, :].reshape(C, 8192)], axis=0)

        in_maps.append(dict(
            brow=brow,
            spt8=spt8h,
            u5d=u5h,
            gseg=np.ascontiguousarray(gsegh.astype(BF)),
            gmask=np.ascontiguousarray(gmaskh.astype(BF)),
            bigwa=np.ascontiguousarray(bigwa.astype(BF)),
            bigwb=bigwb,
            constf=np.ascontiguousarray(constf),
            xb2=np.ascontiguousarray(xb2.astype(BF)),
        ))

    res = run_bass_kernel_spmd(_NC, in_maps, list(range(NCORES)))

    out = np.empty((B, C, H, W), np.float32)
    for c in range(NCORES):
        b, half = divmod(c, 2)
        h0 = half * ROWS
        out[b, :, h0:h0 + ROWS, :] = res.results[c]["out"].astype(
            np.float32).reshape(C, ROWS, W)
    return out


# revision 50
# speedup vs baseline: 1.0778x; 1.0103x over previous
"""Trainium2 Bass kernel for nn_Decoder_22196390985918 (SPADE-style decoder).

Sharding: 8 cores = (batch b in 0..3) x (H-half in 0..1). Each core computes
out[b, :, h0:h0+64, :] for h0 = 64*(core%2).

v2 restructure vs baseline:
- All weights host-side pre-transposed into final lhsT layouts, blend
  (sigmoid) factors folded in on host, cast to bf16, packed into 2 big
  DMA blobs + 1 small fp32 const blob -> ~6 input DMAs total instead of ~60.
- x loaded once as bf16 [128, 8192]: partitions 0-63 = own 64 rows
  (channel-major, reused by the epilogue), 64-127 = other half (stats only).
  Instance-norm stats via one multi-chunk bn_stats; halves combined with
  plain [64]-partition tensor ops (no transposes / DMAs).
- Region-priority mask fused: sel = (cnt == 0) * seg as one DVE
  scalar_tensor_tensor reading cnt straight from PSUM.
- sel45 partition order (j, ty, tx) so the G table rearrange is ONE
  SBUF->SBUF DMA. u5 = kron(tril, eye(9)) accordingly.
- Epilogue: out = (x - mu) * A + B with A = (psum_g + bias_g)*rstd via one
  ACT op, B = psum_b + bias_b via another; no PE broadcast matmul.
- Main conv loop starts as soon as selG is ready (~20us) and overlaps all
  remaining DMA.
"""
import numpy as np
import ml_dtypes

import concourse.bacc as bacc
import concourse.bass as bass
import concourse.mybir as mybir
import concourse.tile as tile
from concourse.bass_utils import run_bass_kernel_spmd

dt = mybir.dt
F32 = dt.float32
BF16 = dt.bfloat16
AF = mybir.ActivationFunctionType
ALU = mybir.AluOpType
BF = ml_dtypes.bfloat16

B, C, H, W, F, L, NH = 4, 64, 128, 128, 5, 512, 128
GW = 130                    # padded grid width  (image col = grid col - 1)
SR = 66                     # seg/sel/actv grid rows (image row = h0 - 1 + r)
MR = 68                     # mask grid rows (image row = h0 - 2 + r)
SEG_N = SR * GW             # 8580
MASK_N = MR * GW            # 8840
GLS = SEG_N + 48            # seg grid line length (incl. u5 tail)
GLM = MASK_N                # mask grid line length
ROWS = 64                   # output rows per core
NCH = 16                    # main conv chunks (4 rows x 128 cols, N=512)
ACH = 22                    # shared conv chunks (3 rows x 128 cols, N=384)
NCORES = 8

# bigwa layout (per-partition elem offsets): fcw [5*4*512] then codes [5*512]
OFF_FCW = 0
OFF_CODES = 5 * 4 * 512     # 10240
BWA = OFF_CODES + 5 * 512   # 12800
# bigwb layout: wct [4*9*128] then spT [9*128] then sswT [128]
OFF_WCT = 0
OFF_SPT = 4 * 9 * 128       # 4608
OFF_SSW = OFF_SPT + 9 * 128  # 5760
BWB = OFF_SSW + 128         # 5888
# constf layout (fp32): 0 biasg, 1 ssb, 2-3 hal, 4-23 fcbt, 24 biasb64
CF = 25
# u5 lives in the grids blob tail on partitions 0..44
OFF_U5 = SEG_N              # 8580..8625


def _build_nc():
    nc = bacc.Bacc()

    gseg_d = nc.dram_tensor("gseg", [46, SEG_N], BF16, kind="ExternalInput")
    gmask_d = nc.dram_tensor("gmask", [27, MASK_N], BF16,
                             kind="ExternalInput")
    u5_d = nc.dram_tensor("u5d", [45, 48], BF16, kind="ExternalInput")
    brow_d = nc.dram_tensor("brow", [1, 128], BF16, kind="ExternalInput")
    spt8_d = nc.dram_tensor("spt8", [128, 1152], dt.float8e4,
                            kind="ExternalInput")
    bigwa_d = nc.dram_tensor("bigwa", [128, BWA], BF16, kind="ExternalInput")
    bigwb_d = nc.dram_tensor("bigwb", [128, BWB], BF16, kind="ExternalInput")
    constf_d = nc.dram_tensor("constf", [128, CF], F32, kind="ExternalInput")
    xb2_d = nc.dram_tensor("xb2", [128, 8192], BF16, kind="ExternalInput")
    out_d = nc.dram_tensor("out", [C, 4, 4 * 512], BF16, kind="ExternalOutput")

    with tile.TileContext(nc) as tc:
        with (
            tc.tile_pool(name="const", bufs=1) as cst,
            tc.tile_pool(name="mus", bufs=2) as musp,
            tc.tile_pool(name="gb", bufs=3) as gbp,
            tc.tile_pool(name="ep", bufs=3) as epp,
            tc.tile_pool(name="ot", bufs=2) as otp,
            tc.tile_pool(name="pmain", bufs=3, space="PSUM") as pmain,
            tc.tile_pool(name="paux", bufs=3, space="PSUM") as paux,
            tc.tile_pool(name="pg", bufs=2, space="PSUM") as pgp,
        ):
            # ---- input DMAs ---------------------------------------------
            segc = cst.tile([16, SEG_N + 2 * GW + 4], BF16)
            nc.sync.dma_start(out=segc[:], in_=segc_d[:])
            maskc = cst.tile([9, MASK_N + 2 * GW + 4], BF16)
            nc.sync.dma_start(out=maskc[:], in_=maskc_d[:])
            u5t = cst.tile([45, 48], BF16)
            nc.sync.dma_start(out=u5t[:], in_=u5_d[:])
            spt8 = cst.tile([128, 4, 2, 128], dt.float8e4)
            spt8b = cst.tile([128, 128], dt.float8e4)
            constf = cst.tile([128, CF], F32)
            nc.gpsimd.dma_start(out=constf[:], in_=constf_d[:])
            bigwa = cst.tile([128, BWA], BF16)
            nc.gpsimd.dma_start(out=bigwa[:], in_=bigwa_d[:])
            bigwb = cst.tile([128, BWB], BF16)
            nc.sync.dma_start(out=bigwb[:], in_=bigwb_d[:])
            xb2 = cst.tile([128, 8192], BF16)
            nc.gpsimd.dma_start(out=xb2[:], in_=xb2_d[:])

            # on-chip 9-shift replication (vector DMA queue, SBUF->SBUF)
            gseg = cst.tile([46, SEG_N], BF16)
            sca = segc[:]
            nc.vector.dma_start(
                out=gseg[0:45, :],
                in_=bass.AP(tensor=sca.tensor, offset=sca.offset,
                            ap=[sca.ap[0][:1] + [5], [GW, 3], [1, 3],
                                [1, SEG_N]]))
            nc.vector.dma_start(out=gseg[45:46, :], in_=segc[5:6, 0:SEG_N])
            gmask = cst.tile([27, MASK_N], BF16)
            mca = maskc[:]
            nc.vector.dma_start(
                out=gmask[:],
                in_=bass.AP(tensor=mca.tensor, offset=mca.offset,
                            ap=[mca.ap[0][:1] + [3], [GW, 3], [1, 3],
                                [1, MASK_N]]))

            sel45 = gseg[0:45, 0:SEG_N]
            u5 = u5t[:, 0:45]
            mask27 = gmask[:, 0:MASK_N]
            epst = cst.tile([128, 1], F32)
            nc.gpsimd.memset(epst[:], 1e-5)
            rstdall = cst.tile([128, 1], F32)
            nc.gpsimd.memset(rstdall[64:128, :], 1.0 / 64.0)
            zt = cst.tile([128, 1], F32)
            nc.gpsimd.memset(zt[:], 0.0)
            biasg = constf[:, 0:1]
            ssb = constf[:, 1:2]
            hal = constf[:, 2:4]
            fcbt = constf[:, 4:24].rearrange("p (k j) -> p k j", k=4)
            biasb64 = constf[0:64, 24:25]

            # ---- region masks: cnt (PE) -> sel = (cnt==0)*seg (DVE) -----
            segchunks = []
            off = 0
            while off < SEG_N:
                n = min(512, SEG_N - off)
                segchunks.append((off, n))
                off += n
            for ci, (off, n) in enumerate(segchunks):
                pc = paux.tile([45, 512], F32, tag="aux")
                nc.tensor.matmul(pc[:, 0:n], u5, sel45[:, off:off + n],
                                 start=True, stop=True)
                if ci % 3 == 2:
                    nc.vector.tensor_scalar_max(sel45[:, off:off + n],
                                                pc[:, 0:n], 0.0)
                else:
                    nc.scalar.activation(sel45[:, off:off + n], pc[:, 0:n],
                                         AF.Relu, bias=zt[0:45, :])

            # ---- shared conv (mask 3 -> NH) + actv assembly -------------
            actv = cst.tile([NH, SR, GW], dt.float8e4)
            bord = actv[:, :, 0:1]
            nc.gpsimd.memset(
                bass.AP(tensor=bord.tensor, offset=bord.offset,
                        ap=[bord.ap[0], [GW, SR], [GW - 1, 2]]), 0.0)
            sswT = bigwb[0:27, OFF_SSW:OFF_SSW + 128]
            m3 = mask27.rearrange("p (r c) -> p r c", c=GW)
            for a in range(ACH):
                r = 3 * a
                psh = paux.tile([NH, 3, 128], F32, tag="aux")
                nc.tensor.matmul(psh[:], sswT, m3[:, r:r + 3, 0:128],
                                 start=True, stop=True)
                if a % 3 == 2:
                    nc.vector.tensor_scalar(actv[:, r:r + 3, 1:129], psh[:],
                                            ssb, 0.0, op0=ALU.add, op1=ALU.max)
                else:
                    nc.scalar.activation(actv[:, r:r + 3, 1:129], psh[:],
                                         AF.Relu, bias=ssb, scale=1.0)
            nc.scalar.activation(actv[:, 0, :], actv[:, 0, :], AF.Identity,
                                  bias=zt[:], scale=hal[:, 0:1])
            nc.scalar.activation(actv[:, SR - 1, :], actv[:, SR - 1, :],
                                 AF.Identity, bias=zt[:], scale=hal[:, 1:2])

            # ---- mu path: z = fcb + sum_l fcw*codes (DVE TTR), relu -----
            z_sb = cst.tile([128, 4, F], F32)
            for j in range(F):
                cview = bigwa[:, OFF_CODES + j * L:OFF_CODES + (j + 1) * L]
                for kb in range(4):
                    fview = bigwa[:, OFF_FCW + (j * 4 + kb) * L:
                                  OFF_FCW + (j * 4 + kb + 1) * L]
                    mus = musp.tile([128, L], BF16, tag="mus")
                    nc.vector.scalar_tensor_tensor(
                        out=mus[:], in0=fview, scalar=1.0, in1=cview,
                        op0=ALU.mult, op1=ALU.mult,
                        accum_out=z_sb[:, kb, j:j + 1])
            nc.vector.tensor_add(z_sb[:], z_sb[:], fcbt[:])
            muT = cst.tile([128, 4, F], BF16)
            nc.vector.tensor_scalar_max(muT[:], z_sb[:], 0.0)

            # ---- G tables -> selG (one rearrange DMA) -------------------
            gstage = cst.tile([F, 9, 128], BF16)
            for g in range(3):
                gps = pgp.tile([F, 3, 128], F32, tag="gps")
                for kb in range(4):
                    wview = bigwb[:, OFF_WCT + kb * 1152 + g * 384:
                                  OFF_WCT + kb * 1152 + (g + 1) * 384]
                    nc.tensor.matmul(gps[:], muT[:, kb, :],
                                     wview.rearrange("p (t c) -> p t c", t=3),
                                     start=(kb == 0), stop=(kb == 3))
                nc.scalar.activation(gstage[:, 3 * g:3 * g + 3, :], gps[:], AF.Copy)
            selG = cst.tile([46, 128], BF16)
            for ty in range(3):
                nc.scalar.dma_start(out=selG[15 * ty:15 * ty + 15, :],
                                    in_=gstage[:, 3 * ty:3 * ty + 3, :])
            nc.scalar.dma_start(out=selG[45:46, :], in_=brow_d[:])

            # ---- instance-norm stats ------------------------------------
            st = cst.tile([128, 16, 6], F32)
            for q in range(16):
                nc.vector.bn_stats(out=st[:, q, :],
                                   in_=xb2[:, q * 512:(q + 1) * 512])
            mv = cst.tile([128, 2], F32)
            nc.vector.bn_aggr(out=mv[:], in_=st[:])
            mvm = cst.tile([128, 2], F32)
            nc.vector.tensor_copy(mvm[:, 0:1], mv[:, 0:1])
            nc.vector.scalar_tensor_tensor(
                out=mvm[:, 1:2], in0=mv[:, 0:1], scalar=mv[:, 0:1], in1=mv[:, 1:2],
                op0=ALU.mult, op1=ALU.add)
            oth = cst.tile([64, 2], F32)
            nc.vector.stream_shuffle(oth[:], mvm[64:128, :], list(range(32)))
            mus_ = cst.tile([64, 1], F32)
            nc.vector.tensor_add(mus_[:], mvm[0:64, 0:1], oth[:, 0:1])
            m2s = cst.tile([64, 1], F32)
            nc.vector.tensor_add(m2s[:], mvm[0:64, 1:2], oth[:, 1:2])
            muc = cst.tile([64, 1], F32)
            nc.vector.tensor_scalar_mul(muc[:], mus_[:], 0.5)
            mu2 = cst.tile([64, 1], F32)
            nc.vector.tensor_mul(mu2[:], muc[:], muc[:])
            varc = cst.tile([64, 1], F32)
            nc.vector.scalar_tensor_tensor(
                out=varc[:], in0=m2s[:], scalar=0.5, in1=mu2[:],
                op0=ALU.mult, op1=ALU.subtract)
            ve = cst.tile([64, 1], F32)
            nc.vector.tensor_scalar_add(ve[:], varc[:], 1e-5)
            ri = cst.tile([64, 1], dt.int32)
            nc.vector.tensor_scalar(ri[:], ve[:].bitcast(dt.int32),
                                    1, None, op0=ALU.arith_shift_right)
            nc.vector.tensor_scalar(ri[:], ri[:], 0x5f3759df, -1,
                                    op0=ALU.subtract, op1=ALU.mult)
            r0 = cst.tile([64, 1], F32)
            nc.vector.tensor_copy(r0[:], ri[:].bitcast(F32))
            rstd = cst.tile([64, 1], F32)
            ra = cst.tile([64, 1], F32)
            rb = cst.tile([64, 1], F32)
            for it in range(2):
                nc.vector.tensor_mul(ra[:], r0[:], r0[:])
                nc.vector.tensor_mul(rb[:], ra[:], ve[:])
                nc.vector.tensor_scalar(rb[:], rb[:], -0.5, 1.5,
                                        op0=ALU.mult, op1=ALU.add)
                nc.vector.tensor_mul(r0[:], r0[:], rb[:])
            nc.vector.tensor_copy(rstd[:], r0[:])
            rstd64 = cst.tile([64, 1], F32)
            nc.vector.tensor_scalar_mul(rstd64[:], rstd[:], 1.0 / 64.0)
            nc.vector.tensor_copy(rstdall[0:64, :], rstd64[:])

            # ---- main conv + epilogue (epilogue one chunk behind) -------
            s3 = gseg[:].rearrange("p (r c) -> p r c", c=GW)
            a3 = actv[:]
            pms = {}
            ots = {}

            aoff = [ty * GW + tx for ty in range(3) for tx in range(3)]
            a8 = actv[:]

            def conv_chunk(i):
                pm = pmain.tile([128, 4, 128], F32, tag="pm", name=f"pm_{i}")
                pms[i] = pm
                for P in range(4):
                    o0, o1 = aoff[2 * P], aoff[2 * P + 1]
                    rhs = bass.AP(
                        tensor=a8.tensor,
                        offset=a8.offset + 4 * i * GW + o0,
                        ap=[a8.ap[0], [o1 - o0, 2], [GW, 4], [1, 128]])
                    nc.tensor.matmul(pm[:], spt8[:, P, :, :], rhs,
                                     start=(P == 0), stop=False,
                                     perf_mode=mybir.MatmulPerfMode.DoubleRow,
                                     skip_group_check=True)
                rhs8 = bass.AP(tensor=a8.tensor,
                               offset=a8.offset + 4 * i * GW + aoff[8],
                               ap=[a8.ap[0], [GW, 4], [1, 128]])
                nc.tensor.matmul(pm[:], spt8b[:], rhs8, start=False, stop=False,
                                 skip_group_check=True)
                nc.tensor.matmul(pm[:], selG[:], s3[:, 4 * i:4 * i + 4, 0:128],
                                 start=False, stop=True, skip_group_check=True)

            def epi_chunk(i):
                pm = pms.pop(i)
                if i % 4 == 0:
                    ots[i // 4] = otp.tile([64, 4, 4, 128], BF16, tag="ot",
                                           name=f"ot_{i // 4}")
                ot = ots[i // 4]
                gt = gbp.tile([128, 4, 128], F32, tag="gb")
                nc.scalar.activation(gt[:], pm[:], AF.Identity,
                                     bias=zt[:], scale=rstdall[:])
                ga = gt[0:64, :, :]
                bbm = epp.tile([64, 4, 128], F32, tag="ep")
                nc.vector.stream_shuffle(bbm[:], gt[64:128, :, :], list(range(32)))
                xa = epp.tile([64, 4, 128], F32, tag="ep")
                nc.vector.scalar_tensor_tensor(
                    out=xa[:], in0=xb2[0:64, i * 512:(i + 1) * 512].rearrange(
                        "p (r w) -> p r w", r=4),
                    scalar=muc[:], in1=ga, op0=ALU.subtract,
                    op1=ALU.mult)
                nc.gpsimd.tensor_add(ot[:, i % 4, :, :], xa[:], bbm[:])
                if i % 4 == 3:
                    q = i // 4
                    nc.sync.dma_start(
                        out=out_d[:, q, :],
                        in_=ots.pop(q)[:].rearrange("c k r w -> c (k r w)"))

            conv_chunk(0)
            for i in range(1, NCH):
                conv_chunk(i)
                epi_chunk(i - 1)
            epi_chunk(NCH - 1)

    nc.finalize()
    return nc


_NC = None


def kernel(**inputs):
    global _NC
    x = np.asarray(inputs["x"], dtype=np.float32)
    segmap = np.asarray(inputs["segmap"], dtype=np.float32)
    codes_vector = np.asarray(inputs["codes_vector"], dtype=np.float32)
    mask = np.asarray(inputs["mask"], dtype=np.float32)
    fc_w = np.asarray(inputs["fc_w"], dtype=np.float32)
    fc_b = np.asarray(inputs["fc_b"], dtype=np.float32)
    cgw = np.asarray(inputs["conv_gamma_w"], dtype=np.float32)
    cgb = np.asarray(inputs["conv_gamma_b"], dtype=np.float32)
    cbw = np.asarray(inputs["conv_beta_w"], dtype=np.float32)
    cbb = np.asarray(inputs["conv_beta_b"], dtype=np.float32)
    ssw = np.asarray(inputs["spade_shared_w"], dtype=np.float32)
    ssb = np.asarray(inputs["spade_shared_b"], dtype=np.float32)
    sgw = np.asarray(inputs["spade_gamma_w"], dtype=np.float32)
    sgb = np.asarray(inputs["spade_gamma_b"], dtype=np.float32)
    sbw = np.asarray(inputs["spade_beta_w"], dtype=np.float32)
    sbb = np.asarray(inputs["spade_beta_b"], dtype=np.float32)
    bg = float(np.asarray(inputs["blending_gamma"]).reshape(-1)[0])
    bb_ = float(np.asarray(inputs["blending_beta"]).reshape(-1)[0])

    if _NC is None:
        _NC = _build_nc()

    ga = 1.0 / (1.0 + np.exp(-bg))
    ba = 1.0 / (1.0 + np.exp(-bb_))

    # bigwb: wct | spT | sswT  (shared across cores)
    bigwb = np.zeros((128, BWB), np.float32)
    # wct[p, kb*1152 + (3ty+tx)*128 + cc] = blend * conv_w[cc, kb*128+p, ty, tx]
    cw = np.concatenate([cgw * ga, cbw * ba], axis=0) * 64.0   # [128, 512, 3, 3]
    wct = cw.reshape(128, 4, 128, 9).transpose(2, 1, 3, 0)     # [p, kb, t, cc]
    bigwb[:, OFF_WCT:OFF_SPT] = wct.reshape(128, 4608)
    sw = np.concatenate([sgw * (1 - ga), sbw * (1 - ba)], axis=0)  # [128, NH, 3, 3]
    spT = sw.reshape(128, 128, 9).transpose(1, 2, 0) * 64.0    # [nh, t, cc]
    spt8h = np.zeros((128, 1152), np.float32)
    spt8h[:, 0:1024] = spT[:, 0:8, :].reshape(128, 1024)
    spt8h[:, 1024:1152] = spT[:, 8, :]
    spt8h = np.ascontiguousarray(spt8h.astype(ml_dtypes.float8_e4m3))
    # sswT[9ty+3c+tx, nh] = ssw[nh, c, ty, tx] (on-chip mask replicate order)
    sswT = ssw.transpose(2, 1, 3, 0).reshape(27, 128)
    bigwb[0:27, OFF_SSW:OFF_SSW + 128] = sswT
    bigwb = bigwb.astype(BF)

    # constf: biasg | ssb | hal | fcbt (hal per-core, rest shared)
    constf_base = np.zeros((128, CF), np.float32)
    constf_base[0:64, 0] = ga * cgb + (1 - ga) * sgb + 1.0
    constf_base[64:128, 0] = ba * cbb + (1 - ba) * sbb
    constf_base[:, 1] = ssb
    constf_base[0:64, 24] = ba * cbb + (1 - ba) * sbb
    # fcbt[p, 4 + kb*5 + j] = fc_b[j, kb*128+p]
    constf_base[:, 4:24] = fc_b.T.reshape(4, 128, F).transpose(1, 0, 2).reshape(128, 20)

    # u5[9j'+t', 9j+t] = (j' > j) * (t'==t)
    u5 = (np.eye(45, dtype=np.float32)
          - np.kron(np.eye(3, dtype=np.float32),
                    np.kron(np.tril(np.ones((F, F), np.float32), -1),
                            np.eye(3, dtype=np.float32))))
    brow = np.zeros((1, 128), np.float32)
    brow[0, 0:64] = (ga * cgb + (1 - ga) * sgb + 1.0) * 64.0
    brow[0, 64:128] = (ba * cbb + (1 - ba) * sbb) * 64.0
    brow = brow.astype(BF)
    u5h = np.zeros((45, 48), np.float32)
    u5h[:, 0:45] = u5
    u5h = np.ascontiguousarray(u5h.astype(BF))

    in_maps = []
    for c in range(NCORES):
        b, half = divmod(c, 2)
        h0 = half * ROWS

        # bigwa: fcw (k-partition, l-free) | codes broadcast (per-batch)
        bigwa = np.zeros((128, BWA), np.float32)
        # fcw_sec[p, (j*4+kb)*512 + l] = fc_w[j, kb*128+p, l]
        bigwa[:, OFF_FCW:OFF_CODES] = (
            fc_w.reshape(F, 4, 128, L).transpose(2, 0, 1, 3).reshape(128, 10240))
        bigwa[:, OFF_CODES:BWA] = np.broadcast_to(
            codes_vector[b].reshape(1, F * L), (128, F * L))

        gsegh = np.ones((46, SEG_N), np.float32)
        gmaskh = np.zeros((27, MASK_N), np.float32)
        segp = np.zeros((F, SR + 2, GW + 2), np.float32)
        r_lo, r_hi = h0 - 1, h0 + ROWS + 1
        s_lo, s_hi = max(r_lo, 0), min(r_hi, H)
        segp[:, s_lo - r_lo:s_hi - r_lo, 1:129] = segmap[b, :, s_lo:s_hi, :]
        for ty in range(3):
            for j in range(F):
                for tx in range(3):
                    gsegh[15 * ty + 3 * j + tx, :] = (
                        segp[j, ty:ty + SR, tx:tx + GW].reshape(-1))
        maskp = np.zeros((3, MR + 2, GW + 2), np.float32)
        m_lo, m_hi = h0 - 2, h0 + ROWS + 2
        ms_lo, ms_hi = max(m_lo, 0), min(m_hi, H)
        maskp[:, ms_lo - m_lo:ms_hi - m_lo, 1:129] = mask[b, :, ms_lo:ms_hi, :]
        for ty in range(3):
            for cc in range(3):
                for tx in range(3):
                    gmaskh[9 * ty + 3 * cc + tx, :] = (
                        maskp[cc, ty:ty + MR, tx:tx + GW].reshape(-1))

        constf = constf_base.copy()
        constf[:, 2] = 0.0 if h0 == 0 else 1.0
        constf[:, 3] = 0.0 if h0 + ROWS == H else 1.0

        xb2 = np.concatenate([
            x[b, :, h0:h0 + ROWS, :].reshape(C, 8192),
            x[b, :, ROWS - h0:ROWS - h0 + ROWS, :].reshape(C, 8192)], axis=0)

        in_maps.append(dict(
            brow=brow,
            spt8=spt8h,
            u5d=u5h,
            gseg=np.ascontiguousarray(gsegh.astype(BF)),
            gmask=np.ascontiguousarray(gmaskh.astype(BF)),
            bigwa=np.ascontiguousarray(bigwa.astype(BF)),
            bigwb=bigwb,
            constf=np.ascontiguousarray(constf),
            xb2=np.ascontiguousarray(xb2.astype(BF)),
        ))

    res = run_bass_kernel_spmd(_NC, in_maps, list(range(NCORES)))

    out = np.empty((B, C, H, W), np.float32)
    for c in range(NCORES):
        b, half = divmod(c, 2)
        h0 = half * ROWS
        out[b, :, h0:h0 + ROWS, :] = res.results[c]["out"].astype(
            np.float32).reshape(C, ROWS, W)
    return out
